# revision 28
# baseline (speedup 1.0000x reference)
"""Trainium2 Bass kernel for the GNN decoder (message passing, cond-layernorm).

Sharding: 8 cores = (batch b in {0,1}) x (pnode quarter q in {0..3}).
Each core owns pnode rows [q*16384, (q+1)*16384) of its batch and every edge
whose receiver lands in that range.  Edges are receiver-sorted on the host and
packed into NG groups of G=104 consecutive segments with a fixed budget of
EPG=512 edge slots per group (padded; pad slots have a zero one-hot row so
they contribute nothing).  The host also pre-gathers sender/receiver features
per edge slot (transposed, bf16) and pre-builds the segment one-hots, so the
device runs a pure dense pipeline: embed MLP -> cond LN -> update MLP ->
cond LN -> one-hot segment-sum matmuls, with the aggregate kept resident in
SBUF for the pnode phase.  LayerNorm rstd uses a fast-inverse-sqrt (bitcast +
Newton) on the vector engine so the scalar engine never leaves the silu
activation-table set.  Cond-norm output affines (1+scale, shift) are folded
into the next matmul's weights on device once per launch.
"""

import numpy as np

import concourse.bass as bass
import concourse.tile as tile
from concourse import bacc
from concourse import mybir

F32 = mybir.dt.float32
BF16 = mybir.dt.bfloat16
I32 = mybir.dt.int32

B, NR, NPTOT, E, F, EIN, H, OUT = 2, 16384, 65536, 262144, 128, 4, 16, 4
EPS = 1e-6
NQ = 4                  # pnode quarters per batch
QP = NPTOT // NQ        # pnodes per core (16384)
G = 104                 # segments per group
EPG = 512               # edge slots per group
NC = EPG // 128         # 128-edge chunks per group (4)
NG = (QP + G - 1) // G  # groups per core (158)
NEP = NG * EPG          # padded edge slots per core
PB = 512                # pnode block width
NPB = QP // PB          # pnode blocks per core (32)
MAGIC = 0x5F3759DF      # fast-inverse-sqrt seed constant

AF = mybir.ActivationFunctionType
ALU = mybir.AluOpType


def _build_nc(skip_bias=False):
    nc = bacc.Bacc("TRN2", target_bir_lowering=False, debug=False)

    def inp(name, shape, dtype=F32):
        return nc.dram_tensor(name, shape, dtype, kind="ExternalInput")

    efT = inp("efT", [EIN, NEP], BF16)
    sfT_d = inp("sfT", [F, NEP], BF16)
    rfT_d = inp("rfT", [F, NEP], BF16)
    oh_d = inp("oh", [128, NG * NC * G], BF16)
    pnT_d = inp("pnT", [F, QP], BF16)
    tau_d = inp("tau", [1, 1])
    m01_d = inp("m01", [1, QP], BF16)

    We1 = inp("We1", [EIN, F], BF16)
    be1 = inp("be1", [F, 1])
    We2 = inp("We2", [F, F], BF16)
    be2 = inp("be2", [1, NC * F], BF16)
    Wu1a = inp("Wu1a", [F, F])
    Wu1b = inp("Wu1b", [F, F], BF16)
    Wu1c = inp("Wu1c", [F, F], BF16)
    bu1 = inp("bu1", [F, 1])
    Wu2 = inp("Wu2", [F, F], BF16)
    bu2 = inp("bu2", [1, NC * F], BF16)
    Wp1n = inp("Wp1n", [F, F], BF16)
    Wp1g = inp("Wp1g", [F, F])
    bp1 = inp("bp1", [F, 1])
    Wp2 = inp("Wp2", [F, F], BF16)
    bp2 = inp("bp2", [1, NC * F], BF16)
    Wo1 = inp("Wo1", [F, F])
    bo1 = inp("bo1", [F, 1])
    Wo2 = inp("Wo2", [F, OUT], BF16)
    bo2 = inp("bo2", [1, OUT], BF16)
    # conditioning nets: e(dge embed), u(pdate), p(node).  r is dead code.
    cond_w = {}
    for k in ("e", "u", "p"):
        cond_w[k] = (
            inp(f"C{k}1", [1, H]),
            inp(f"c{k}1", [H, 1]),
            inp(f"C{k}2a", [H, F]),     # scale half of C2
            inp(f"C{k}2b", [H, F]),     # shift half of C2
            inp(f"c{k}2a1", [F, 1]),    # c2[:F] + 1.0
            inp(f"c{k}2b", [F, 1]),     # c2[F:]
        )

    outT = nc.dram_tensor("outT", [OUT, QP], F32, kind="ExternalOutput")

    from contextlib import ExitStack

    with tile.TileContext(nc) as tc, ExitStack() as ctx:
        singles = ctx.enter_context(tc.tile_pool(name="singles", bufs=1))
        ld = ctx.enter_context(tc.tile_pool(name="ld", bufs=3))
        work = ctx.enter_context(tc.tile_pool(name="work", bufs=4))
        small = ctx.enter_context(tc.tile_pool(name="small", bufs=4))
        psB = ctx.enter_context(tc.tile_pool(name="psB", bufs=2, space="PSUM"))
        psA = ctx.enter_context(tc.tile_pool(name="psA", bufs=4, space="PSUM"))
        psS = ctx.enter_context(tc.tile_pool(name="psS", bufs=2, space="PSUM"))

        # ---------- constants & resident tables ----------
        def load(name, dram, shape, dtype=F32):
            t = singles.tile(shape, dtype, tag=name)
            nc.sync.dma_start(out=t[:], in_=dram[:])
            return t

        sWe1 = load("We1", We1, [EIN, F], BF16)
        sbe1 = load("be1", be1, [F, 1])
        sWe2 = load("We2", We2, [F, F], BF16)
        sbe2 = load("be2", be2, [1, NC * F], BF16)
        sWu1a = load("Wu1a", Wu1a, [F, F])
        sWu1b = load("Wu1b", Wu1b, [F, F], BF16)
        sWu1c = load("Wu1c", Wu1c, [F, F], BF16)
        sbu1 = load("bu1", bu1, [F, 1])
        sWu2 = load("Wu2", Wu2, [F, F], BF16)
        sbu2 = load("bu2", bu2, [1, NC * F], BF16)
        sWp1n = load("Wp1n", Wp1n, [F, F], BF16)
        sWp1g = load("Wp1g", Wp1g, [F, F])
        sbp1 = load("bp1", bp1, [F, 1])
        sWp2 = load("Wp2", Wp2, [F, F], BF16)
        sbp2 = load("bp2", bp2, [1, NC * F], BF16)
        sWo1 = load("Wo1", Wo1, [F, F])
        sbo1 = load("bo1", bo1, [F, 1])
        sWo2 = load("Wo2", Wo2, [F, OUT], BF16)
        sbo2 = load("bo2", bo2, [1, OUT], BF16)
        stau = load("tau", tau_d, [1, 1])
        sm01 = load("m01", m01_d, [1, QP], BF16)

        from concourse.masks import make_identity

        ident = singles.tile([128, 128], F32, tag="ident")
        make_identity(nc, ident[:])
        ident16 = singles.tile([128, 128], BF16, tag="ident16")
        nc.vector.tensor_copy(out=ident16[:], in_=ident[:])
        ones_r = singles.tile([1, PB], BF16, tag="ones_r")
        nc.vector.memset(ones_r[:], 1.0)
        ones_r32 = singles.tile([1, 128], F32, tag="ones_r32")
        nc.vector.memset(ones_r32[:], 1.0)

        # resident aggregate accumulators (segment sums, transposed)
        aggSB1 = singles.tile([128, NG * G], BF16, tag="aggSB1")
        aggSB2 = singles.tile([128, NG * G], BF16, tag="aggSB2")

        # ---------- conditioning nets (tau -> scale/shift) + weight folds ----
        cvec = {}
        for k in ("e", "u", "p"):
            C1, c1, C2a, C2b, c2a1, c2b = cond_w[k]
            sC1 = load(f"C{k}1", C1, [1, H])
            sc1 = load(f"c{k}1", c1, [H, 1])
            sC2a = load(f"C{k}2a", C2a, [H, F])
            sC2b = load(f"C{k}2b", C2b, [H, F])
            sc2a1 = load(f"c{k}2a1", c2a1, [F, 1])
            sc2b = load(f"c{k}2b", c2b, [F, 1])

            ph = psS.tile([H, 1], F32, tag="pS")
            nc.tensor.matmul(ph[:], lhsT=sC1[:], rhs=stau[:], start=True, stop=True)
            hs = small.tile([H, 1], F32, tag=f"hs{k}")
            nc.scalar.activation(hs[:], ph[:], AF.Silu, bias=sc1[:], scale=1.0)

            pscale = psS.tile([F, 1], F32, tag="pS")
            nc.tensor.matmul(pscale[:], lhsT=sC2a[:], rhs=hs[:], start=True, stop=True)
            s1p = singles.tile([F, 1], F32, tag=f"s1p{k}")
            nc.vector.tensor_scalar(
                out=s1p[:], in0=pscale[:], scalar1=sc2a1[:], scalar2=None, op0=ALU.add
            )
            pshift = psS.tile([F, 1], F32, tag="pS")
            nc.tensor.matmul(pshift[:], lhsT=sC2b[:], rhs=hs[:], start=True, stop=True)
            shift = singles.tile([F, 1], F32, tag=f"shift{k}")
            nc.vector.tensor_scalar(
                out=shift[:], in0=pshift[:], scalar1=sc2b[:], scalar2=None, op0=ALU.add
            )
            cvec[k] = (s1p, shift)

        s1pe, shifte = cvec["e"]
        s1pu, shiftu = cvec["u"]
        s1pp, shiftp = cvec["p"]

        # fold cond-norm affines into downstream weights
        fWu1a = singles.tile([F, F], BF16, tag="fWu1a")
        nc.vector.tensor_tensor(
            out=fWu1a[:], in0=sWu1a[:], in1=s1pe[:].to_broadcast([F, F]), op=ALU.mult
        )
        pbu1 = psS.tile([F, 1], F32, tag="pS")
        nc.tensor.matmul(pbu1[:], lhsT=sWu1a[:], rhs=shifte[:], start=True, stop=True)
        fbu1 = singles.tile([F, 1], F32, tag="fbu1")
        nc.vector.tensor_scalar(
            out=fbu1[:], in0=pbu1[:], scalar1=sbu1[:], scalar2=None, op0=ALU.add
        )

        fWp1ge = singles.tile([F, F], BF16, tag="fWp1ge")
        nc.vector.tensor_tensor(
            out=fWp1ge[:], in0=sWp1g[:], in1=s1pe[:].to_broadcast([F, F]), op=ALU.mult
        )
        fWp1gu = singles.tile([F, F], BF16, tag="fWp1gu")
        nc.vector.tensor_tensor(
            out=fWp1gu[:], in0=sWp1g[:], in1=s1pu[:].to_broadcast([F, F]), op=ALU.mult
        )
        shifteu = small.tile([F, 1], F32, tag="shifteu")
        nc.vector.tensor_tensor(
            out=shifteu[:], in0=shifte[:], in1=shiftu[:], op=ALU.add
        )
        pbpe = psS.tile([1, F], F32, tag="pS")
        nc.tensor.matmul(pbpe[:], lhsT=shifteu[:], rhs=sWp1g[:], start=True, stop=True)
        bpe_row = singles.tile([1, F], BF16, tag="bpe_row")
        nc.vector.tensor_copy(out=bpe_row[:], in_=pbpe[:])

        fWo1 = singles.tile([F, F], BF16, tag="fWo1")
        nc.vector.tensor_tensor(
            out=fWo1[:], in0=sWo1[:], in1=s1pp[:].to_broadcast([F, F]), op=ALU.mult
        )
        sWo116 = singles.tile([F, F], BF16, tag="sWo116")
        nc.vector.tensor_copy(out=sWo116[:], in_=sWo1[:])
        pbo1 = psS.tile([F, 1], F32, tag="pS")
        nc.tensor.matmul(pbo1[:], lhsT=sWo1[:], rhs=shiftp[:], start=True, stop=True)
        fbo1 = singles.tile([F, 1], F32, tag="fbo1")
        nc.vector.tensor_scalar(
            out=fbo1[:], in0=pbo1[:], scalar1=sbo1[:], scalar2=None, op0=ALU.add
        )

        # LN bn-stats into a shared mv view (cols of a [128, 8, 2] tile)
        def ln_bn(psum4, nch, mv_view, tag):
            for c in range(nch):
                st6 = small.tile([128, 6], F32, tag=f"st{tag}", name="st6")
                nc.vector.bn_stats(out=st6[:], in_=psum4[:, c, :])
                nc.vector.bn_aggr(out=mv_view[:, c, :], in_=st6[:])

        # combined fast-inverse-sqrt over a [128, n, 2] mv tile -> rstd/negms
        def fisr_n(mvc, n, tag, iters=1):
            a = small.tile([128, n], F32, tag=f"a{tag}", name="a")
            nc.vector.tensor_scalar(
                out=a[:], in0=mvc[:, :, 1], scalar1=EPS, scalar2=None, op0=ALU.add
            )
            bi = small.tile([128, n], I32, tag=f"bi{tag}", name="bi")
            nc.vector.tensor_scalar(
                out=bi[:], in0=a[:].bitcast(I32), scalar1=1, scalar2=None,
                op0=ALU.arith_shift_right,
            )
            y0 = small.tile([128, n], F32, tag=f"y0{tag}", name="y0")
            nc.vector.tensor_scalar(
                out=y0[:].bitcast(I32), in0=bi[:], scalar1=-1, scalar2=MAGIC,
                op0=ALU.mult, op1=ALU.add,
            )
            ha = small.tile([128, n], F32, tag=f"ha{tag}", name="ha")
            nc.vector.tensor_scalar(
                out=ha[:], in0=a[:], scalar1=-0.5, scalar2=None, op0=ALU.mult
            )
            y = y0
            for it in range(iters):
                yy = small.tile([128, n], F32, tag=f"yy{tag}{it}", name="yy")
                nc.vector.tensor_tensor(out=yy[:], in0=y[:], in1=y[:], op=ALU.mult)
                hyy = small.tile([128, n], F32, tag=f"hy{tag}{it}", name="hyy")
                nc.vector.tensor_tensor(out=hyy[:], in0=yy[:], in1=ha[:], op=ALU.mult)
                yn = small.tile([128, n], F32, tag=f"yn{tag}{it}", name="yn")
                nc.vector.scalar_tensor_tensor(
                    out=yn[:], in0=hyy[:], scalar=1.5, in1=y[:],
                    op0=ALU.add, op1=ALU.mult,
                )
                y = yn
            negms = small.tile([128, n], F32, tag=f"nm{tag}", name="negms")
            nc.vector.scalar_tensor_tensor(
                out=negms[:], in0=mvc[:, :, 0], scalar=-1.0, in1=y[:],
                op0=ALU.mult, op1=ALU.mult,
            )
            return y, negms

        # LayerNorm stats helper: psum4 [128, nch, 128] -> (mv, rstd, negms)
        #   mv[:, c, 0] = mean, rstd = 1/sqrt(var+eps), negms = -mean*rstd
        # bn runs on vector (PSUM reads); the rsqrt Newton chain runs on the
        # otherwise-idle gpsimd engine (SBUF-only tiles).
        def ln_stats(psum4, nch, tag, iters=1):
            mv = small.tile([128, nch, 2], F32, tag=f"mv{tag}")
            for c in range(nch):
                st6 = small.tile([128, 6], F32, tag=f"st{tag}")
                nc.vector.bn_stats(out=st6[:], in_=psum4[:, c, :])
                nc.vector.bn_aggr(out=mv[:, c, :], in_=st6[:])
            a = small.tile([128, nch], F32, tag=f"a{tag}")
            nc.vector.tensor_scalar(
                out=a[:], in0=mv[:, :, 1], scalar1=EPS, scalar2=None, op0=ALU.add
            )
            bi = small.tile([128, nch], I32, tag=f"bi{tag}")
            nc.vector.tensor_scalar(
                out=bi[:], in0=a[:].bitcast(I32), scalar1=1, scalar2=None,
                op0=ALU.arith_shift_right,
            )
            y0 = small.tile([128, nch], F32, tag=f"y0{tag}")
            nc.vector.tensor_scalar(
                out=y0[:].bitcast(I32), in0=bi[:], scalar1=-1, scalar2=MAGIC,
                op0=ALU.mult, op1=ALU.add,
            )
            ha = small.tile([128, nch], F32, tag=f"ha{tag}")
            nc.vector.tensor_scalar(
                out=ha[:], in0=a[:], scalar1=-0.5, scalar2=None, op0=ALU.mult
            )
            y = y0
            for it in range(iters):
                yy = small.tile([128, nch], F32, tag=f"yy{tag}{it}")
                nc.vector.tensor_tensor(out=yy[:], in0=y[:], in1=y[:], op=ALU.mult)
                hyy = small.tile([128, nch], F32, tag=f"hy{tag}{it}")
                nc.vector.tensor_tensor(out=hyy[:], in0=yy[:], in1=ha[:], op=ALU.mult)
                yn = small.tile([128, nch], F32, tag=f"yn{tag}{it}")
                nc.vector.scalar_tensor_tensor(
                    out=yn[:], in0=hyy[:], scalar=1.5, in1=y[:],
                    op0=ALU.add, op1=ALU.mult,
                )
                y = yn
            negms = small.tile([128, nch], F32, tag=f"nm{tag}")
            nc.vector.scalar_tensor_tensor(
                out=negms[:], in0=mv[:, :, 0], scalar=-1.0, in1=y[:],
                op0=ALU.mult, op1=ALU.mult,
            )
            return mv, y, negms

        # ---------- edge phase (4-stage software pipeline over groups) ------
        # S0: DMA loads   S1: embed MLP + LN-e stats   S2: apply-e, transpose,
        # update MLP + LN-u stats   S3: apply-u, one-hot aggregation.
        # Stages of group g are issued in different loop iterations so every
        # engine's program interleaves adjacent groups (no head-of-line
        # blocking on cross-engine dependencies).

        def edge_s0(g):
            # loads groups g and g+1 in one DMA per stream (g is even)
            hi = min(g + 2, NG)
            w = hi - g
            esl = slice(g * EPG, hi * EPG)
            t = {}
            t["efg"] = ld.tile([EIN, 2 * EPG], BF16, tag="efg", name="efg")
            nc.sync.dma_start(out=t["efg"][:, : w * EPG], in_=efT[:, esl])
            t["sfg"] = ld.tile([128, 2 * EPG], BF16, tag="sfg", name="sfg")
            nc.sync.dma_start(out=t["sfg"][:, : w * EPG], in_=sfT_d[:, esl])
            t["rfg"] = ld.tile([128, 2 * EPG], BF16, tag="rfg", name="rfg")
            nc.sync.dma_start(out=t["rfg"][:, : w * EPG], in_=rfT_d[:, esl])
            t["ohg"] = ld.tile([128, 2 * NC, G], BF16, tag="ohg", name="ohg")
            nc.sync.dma_start(
                out=t["ohg"][:, : w * NC, :],
                in_=oh_d[:, g * NC * G : hi * NC * G],
            )
            return t

        def edge_s1(g, t):
            h = g % 2
            pz1 = psB.tile([128, EPG], F32, tag="pB")
            nc.tensor.matmul(
                pz1[:], lhsT=sWe1[:],
                rhs=t["efg"][:, h * EPG : (h + 1) * EPG], start=True, stop=True,
            )
            y1 = work.tile([128, EPG], BF16, tag="y1")
            nc.scalar.activation(y1[:], pz1[:], AF.Silu, bias=sbe1[:], scale=1.0)

            pz2 = psA.tile([128, NC, 128], F32, tag="pA")
            if not skip_bias:
                nc.tensor.matmul(
                    pz2[:, :, :], lhsT=ones_r[:, :128], rhs=sbe2[:],
                    start=True, stop=False, skip_group_check=True,
                )
            for c in range(NC):
                nc.tensor.matmul(
                    pz2[:, c, :],
                    lhsT=y1[:, c * 128 : (c + 1) * 128],
                    rhs=sWe2[:],
                    start=skip_bias,
                    stop=skip_bias or (c == NC - 1),
                    skip_group_check=True,
                )
            t["pz2"] = pz2
            ln_bn(pz2, NC, t["mve"], "e")

        def edge_s2(g, t):
            pz2 = t["pz2"]
            rs8, nm8 = t["fe"]
            ln1 = work.tile([128, NC, 128], BF16, tag="ln1")
            for c in range(NC):
                nc.scalar.activation(
                    ln1[:, c, :], pz2[:, c, :], AF.Identity,
                    bias=nm8[:, c : c + 1], scale=rs8[:, c : c + 1],
                )
            t["ln1"] = ln1

            ptr = psS.tile([128, NC, 128], BF16, tag="pS")
            for c in range(NC):
                nc.tensor.transpose(ptr[:, c, :], ln1[:, c, :], ident16[:])
            ln1T = work.tile([128, NC // 2, 2, 128], BF16, tag="ln1T")
            nc.vector.tensor_copy(out=ln1T[:, :, 0, :], in_=ptr[:, 0::2, :])
            nc.scalar.activation(ln1T[:, :, 1, :], ptr[:, 1::2, :], AF.Copy)

            pu1 = psB.tile([128, EPG], F32, tag="pB")
            nc.tensor.matmul(
                pu1[:], lhsT=fWu1a[:], rhs=ln1T[:],
                start=True, stop=False,
            )
            h = g % 2
            nc.tensor.matmul(
                pu1[:], lhsT=sWu1b[:],
                rhs=t["sfg"][:, h * EPG : (h + 1) * EPG],
                start=False, stop=False,
            )
            nc.tensor.matmul(
                pu1[:], lhsT=sWu1c[:],
                rhs=t["rfg"][:, h * EPG : (h + 1) * EPG],
                start=False, stop=True,
            )
            yu = work.tile([128, EPG], BF16, tag="yu")
            nc.scalar.activation(yu[:], pu1[:], AF.Silu, bias=fbu1[:], scale=1.0)

            pu2 = psA.tile([128, NC, 128], F32, tag="pA")
            if not skip_bias:
                nc.tensor.matmul(
                    pu2[:, :, :], lhsT=ones_r[:, :128], rhs=sbu2[:],
                    start=True, stop=False, skip_group_check=True,
                )
            for c in range(NC):
                nc.tensor.matmul(
                    pu2[:, c, :],
                    lhsT=yu[:, c * 128 : (c + 1) * 128],
                    rhs=sWu2[:],
                    start=skip_bias,
                    stop=skip_bias or (c == NC - 1),
                    skip_group_check=True,
                )
            t["pu2"] = pu2
            ln_bn(pu2, NC, t["mvu"], "u")

        def edge_s3(g, t):
            pu2 = t["pu2"]
            rs8, nm8 = t["fu"]
            mvu = t["mvu"]
            ln1 = t["ln1"]
            ln2 = work.tile([128, NC, 128], BF16, tag="ln2")
            for c in range(NC):
                if c % 2 == 0:
                    nc.scalar.activation(
                        ln2[:, c, :], pu2[:, c, :], AF.Identity,
                        bias=nm8[:, 4 + c : 5 + c], scale=rs8[:, 4 + c : 5 + c],
                    )
                else:
                    nc.vector.tensor_scalar(
                        out=ln2[:, c, :], in0=pu2[:, c, :],
                        scalar1=mvu[:, c, 0:1], scalar2=rs8[:, 4 + c : 5 + c],
                        op0=ALU.subtract, op1=ALU.mult,
                    )

            h = g % 2
            Sps = psS.tile([128, 2 * G], F32, tag="pS")
            for c in range(NC):
                nc.tensor.matmul(
                    Sps[:, 0:G], lhsT=ln1[:, c, :],
                    rhs=t["ohg"][:, h * NC + c, :],
                    start=(c == 0), stop=(c == NC - 1),
                )
            for c in range(NC):
                nc.tensor.matmul(
                    Sps[:, G : 2 * G], lhsT=ln2[:, c, :],
                    rhs=t["ohg"][:, h * NC + c, :],
                    start=(c == 0), stop=(c == NC - 1),
                )
            nc.vector.tensor_copy(
                out=aggSB1[:, g * G : (g + 1) * G], in_=Sps[:, 0:G]
            )
            nc.vector.tensor_copy(
                out=aggSB2[:, g * G : (g + 1) * G], in_=Sps[:, G : 2 * G]
            )

        state = {}
        shared = {}
        for i in range(NG + 3):
            if i < NG and i % 2 == 0:
                shared[i] = edge_s0(i)
                state[i] = dict(shared[i])
                if i + 1 < NG:
                    state[i + 1] = dict(shared[i])
            has_e = 0 <= i - 1 < NG
            has_u = 0 <= i - 2 < NG
            mvc = None
            if has_e or has_u:
                mvc = small.tile([128, 8, 2], F32, tag="mvc", name="mvc")
                if not has_e:
                    nc.vector.memset(mvc[:, 0:4, :], 1.0)
                if not has_u:
                    nc.vector.memset(mvc[:, 4:8, :], 1.0)
            if has_e:
                state[i - 1]["mve"] = mvc[:, 0:4, :]
                edge_s1(i - 1, state[i - 1])
            if has_u:
                state[i - 2]["mvu"] = mvc[:, 4:8, :]
                edge_s2(i - 2, state[i - 2])
            if has_e or has_u:
                r8, n8 = fisr_n(mvc, 8, "c")
                if has_e:
                    state[i - 1]["fe"] = (r8, n8)
                if has_u:
                    state[i - 2]["fu"] = (r8, n8)
            if 0 <= i - 3 < NG:
                edge_s3(i - 3, state[i - 3])
                del state[i - 3]

        # ---------- pnode phase (2-stage software pipeline over blocks) -----
        def pn_s1(j):
            sl = slice(j * PB, (j + 1) * PB)
            t = {"sl": sl}
            pn16b = ld.tile([128, PB], BF16, tag="pn16b")
            nc.sync.dma_start(out=pn16b[:], in_=pnT_d[:, sl])
            t["pn16b"] = pn16b

            pzp = psB.tile([128, PB], F32, tag="pB")
            nc.tensor.matmul(pzp[:], lhsT=sWp1n[:], rhs=pn16b[:], start=True, stop=False)
            nc.tensor.matmul(
                pzp[:], lhsT=fWp1ge[:], rhs=aggSB1[:, sl], start=False, stop=False
            )
            nc.tensor.matmul(
                pzp[:], lhsT=fWp1gu[:], rhs=aggSB2[:, sl], start=False, stop=False
            )
            nc.tensor.matmul(
                pzp[:], lhsT=bpe_row[:], rhs=sm01[:, sl], start=False, stop=True
            )
            yp = work.tile([128, PB], BF16, tag="yu")
            nc.scalar.activation(yp[:], pzp[:], AF.Silu, bias=sbp1[:], scale=1.0)

            pp2 = psA.tile([128, NC, 128], F32, tag="pA")
            if not skip_bias:
                nc.tensor.matmul(
                    pp2[:, :, :], lhsT=ones_r[:, :128], rhs=sbp2[:],
                    start=True, stop=False, skip_group_check=True,
                )
            for c in range(NC):
                nc.tensor.matmul(
                    pp2[:, c, :],
                    lhsT=yp[:, c * 128 : (c + 1) * 128],
                    rhs=sWp2[:],
                    start=skip_bias,
                    stop=skip_bias or (c == NC - 1),
                    skip_group_check=True,
                )
            t["pp2"] = pp2
            mvp = small.tile([128, 4, 2], F32, tag="mvp", name="mvp")
            ln_bn(pp2, NC, mvp[:, :, :], "p")
            t["mvp"] = mvp
            t["fp"] = fisr_n(mvp, 4, "p")
            return t

        def pn_s2(j, t):
            sl = t["sl"]
            pp2 = t["pp2"]
            rsp, nmp = t["fp"]
            lnp = work.tile([128, NC, 128], BF16, tag="ln1")
            for c in range(NC):
                nc.scalar.activation(
                    lnp[:, c, :], pp2[:, c, :], AF.Identity,
                    bias=nmp[:, c : c + 1], scale=rsp[:, c : c + 1],
                )

            ptr2 = psS.tile([128, NC, 128], BF16, tag="pS")
            for c in range(NC):
                nc.tensor.transpose(ptr2[:, c, :], lnp[:, c, :], ident16[:])
            lnpT = work.tile([128, NC // 2, 2, 128], BF16, tag="ln1T")
            nc.vector.tensor_copy(out=lnpT[:, :, 0, :], in_=ptr2[:, 0::2, :])
            nc.scalar.activation(lnpT[:, :, 1, :], ptr2[:, 1::2, :], AF.Copy)

            pzo = psB.tile([128, PB], F32, tag="pB")
            nc.tensor.matmul(
                pzo[:], lhsT=fWo1[:], rhs=lnpT[:],
                start=True, stop=False,
            )
            nc.tensor.matmul(
                pzo[:], lhsT=sWo116[:], rhs=t["pn16b"][:], start=False, stop=True
            )
            yo = work.tile([128, PB], BF16, tag="ln2")
            nc.scalar.activation(yo[:], pzo[:], AF.Silu, bias=fbo1[:], scale=1.0)

            po = psS.tile([OUT, PB], F32, tag="pS")
            nc.tensor.matmul(po[:], lhsT=sWo2[:], rhs=yo[:], start=True, stop=False)
            nc.tensor.matmul(po[:], lhsT=sbo2[:], rhs=ones_r[:], start=False, stop=True)
            oc = work.tile([OUT, PB], F32, tag="oc")
            nc.vector.tensor_copy(out=oc[:], in_=po[:])
            nc.sync.dma_start(out=outT[:, sl], in_=oc[:])

        pstate = {}
        for i in range(NPB + 1):
            if i < NPB:
                pstate[i] = pn_s1(i)
            if 0 <= i - 1 < NPB:
                pn_s2(i - 1, pstate[i - 1])
                del pstate[i - 1]

    nc.compile()
    return nc


def _prep_core(ef_b, snd_b, rcv_b, rn_b, pn_b, tau_b, q):
    import ml_dtypes

    lo = q * QP
    mask = (rcv_b >= lo) & (rcv_b < lo + QP)
    ed = np.nonzero(mask)[0]
    loc = (rcv_b[ed] - lo).astype(np.int64)
    order = np.argsort(loc, kind="stable")
    ed, loc = ed[order], loc[order]
    grp = loc // G
    cnts = np.bincount(grp, minlength=NG)
    assert cnts.max() <= EPG, f"group overflow: {cnts.max()} > {EPG}"
    gstart = np.concatenate([[0], np.cumsum(cnts)[:-1]])
    slot = grp * EPG + (np.arange(len(ed)) - gstart[grp])

    efp = np.zeros((NEP, EIN), np.float32)
    efp[slot] = ef_b[ed]
    sf = np.zeros((NEP, F), np.float32)
    sf[slot] = rn_b[snd_b[ed]]
    rf = np.zeros((NEP, F), np.float32)
    rf[slot] = pn_b[lo + rcv_b[ed] - lo]
    cnt_all = np.bincount(loc, minlength=QP).astype(np.float32)
    ohf = np.zeros((NEP, G), np.float32)
    ohf[slot, loc - grp * G] = 1.0 / cnt_all[loc]
    oh_dev = np.ascontiguousarray(
        ohf.reshape(NG, NC, 128, G).transpose(2, 0, 1, 3).reshape(128, NG * NC * G)
    )

    m01_seg = np.minimum(cnt_all, 1.0)

    pn_q = pn_b[lo : lo + QP]
    bf = ml_dtypes.bfloat16
    return {
        "m01": m01_seg.reshape(1, QP).astype(bf),
        "efT": np.ascontiguousarray(efp.T.astype(bf)),
        "sfT": np.ascontiguousarray(sf.T.astype(bf)),
        "rfT": np.ascontiguousarray(rf.T.astype(bf)),
        "oh": oh_dev.astype(bf),
        "pnT": np.ascontiguousarray(pn_q.T.astype(bf)),
        "tau": tau_b.reshape(1, 1).astype(np.float32),
    }


def _prep_weights(i):
    w = {
        "We1": i["We1"], "be1": i["be1"].reshape(F, 1), "We2": i["We2"],
        "be2": np.tile(i["be2"].reshape(1, F), (1, NC)),
        "Wu1a": i["Wu1"][0:F], "Wu1b": i["Wu1"][F : 2 * F],
        "Wu1c": i["Wu1"][2 * F : 3 * F],
        "bu1": i["bu1"].reshape(F, 1), "Wu2": i["Wu2"],
        "bu2": np.tile(i["bu2"].reshape(1, F), (1, NC)),
        "Wp1n": i["Wp1"][0:F], "Wp1g": i["Wp1"][F : 2 * F],
        "bp1": i["bp1"].reshape(F, 1), "Wp2": i["Wp2"],
        "bp2": np.tile(i["bp2"].reshape(1, F), (1, NC)),
        "Wo1": i["Wo1"], "bo1": i["bo1"].reshape(F, 1), "Wo2": i["Wo2"],
        "bo2": i["bo2"].reshape(1, OUT),
    }
    for k in ("e", "u", "p"):
        C1, c1 = i[f"C{k}1"], i[f"c{k}1"]
        C2, c2 = i[f"C{k}2"], i[f"c{k}2"]
        w[f"C{k}1"] = C1.reshape(1, H)
        w[f"c{k}1"] = c1.reshape(H, 1)
        w[f"C{k}2a"] = np.ascontiguousarray(C2[:, 0:F])
        w[f"C{k}2b"] = np.ascontiguousarray(C2[:, F : 2 * F])
        w[f"c{k}2a1"] = (c2[0:F] + 1.0).reshape(F, 1)
        w[f"c{k}2b"] = c2[F : 2 * F].reshape(F, 1)
    import ml_dtypes

    bf16_keys = {"We1", "We2", "Wu1b", "Wu1c", "Wu2", "Wp1n", "Wp2", "Wo2",
                 "be2", "bu2", "bp2", "bo2"}
    return {
        k: np.ascontiguousarray(
            v, dtype=ml_dtypes.bfloat16 if k in bf16_keys else np.float32
        )
        for k, v in w.items()
    }


_NC_CACHE = {}


def _all_bias_zero(i):
    return all(
        not np.any(np.asarray(i[k]))
        for k in ("be2", "bu2", "bp2")
    )


def build_in_maps(inputs):
    i = {k: np.asarray(v) for k, v in inputs.items()}
    w = _prep_weights(i)
    in_maps = []
    for core in range(8):
        b, q = core // NQ, core % NQ
        m = dict(w)
        m.update(
            _prep_core(
                i["edge_features"][b], i["senders"][b], i["receivers"][b],
                i["rnode_features"][b], i["pnode_features"][b], i["tau"][b], q
            )
        )
        in_maps.append(m)
    return in_maps


def get_nc(skip_bias=False):
    key = ("nc", bool(skip_bias))
    if key not in _NC_CACHE:
        _NC_CACHE[key] = _build_nc(skip_bias=skip_bias)
    return _NC_CACHE[key]


def assemble(results):
    out = np.zeros((B, NPTOT, OUT), np.float32)
    for core in range(8):
        b, q = core // NQ, core % NQ
        out[b, q * QP : (q + 1) * QP, :] = results[core]["outT"].T
    return out


def kernel(**inputs):
    from concourse.bass_utils import run_bass_kernel_spmd

    nc = get_nc(skip_bias=_all_bias_zero(inputs))
    in_maps = build_in_maps(inputs)
    res = run_bass_kernel_spmd(nc, in_maps, list(range(8)))
    return assemble(res.results)


if __name__ == "__main__":
    import reference

    inputs = reference.setup_inputs()
    out = kernel(**{k: np.asarray(v) for k, v in inputs.items()})
    print("out", out.shape, out.dtype)


# revision 29
# speedup vs baseline: 1.4020x; 1.4020x over previous
"""Trainium2 Bass kernel for the GNN decoder (message passing, cond-layernorm).

Sharding: 8 cores = (batch b in {0,1}) x (pnode quarter q in {0..3}).
Each core owns pnode rows [q*16384, (q+1)*16384) of its batch and every edge
whose receiver lands in that range.  Edges are receiver-sorted on the host and
packed into NG groups of G=104 consecutive segments with a fixed budget of
EPG=512 edge slots per group (padded; pad slots have a zero one-hot row so
they contribute nothing).  The host also pre-gathers sender/receiver features
per edge slot (transposed, bf16) and pre-builds the segment one-hots, so the
device runs a pure dense pipeline: embed MLP -> cond LN -> update MLP ->
cond LN -> one-hot segment-sum matmuls, with the aggregate kept resident in
SBUF for the pnode phase.  LayerNorm rstd uses a fast-inverse-sqrt (bitcast +
Newton) on the vector engine so the scalar engine never leaves the silu
activation-table set.  Cond-norm output affines (1+scale, shift) are folded
into the next matmul's weights on device once per launch.
"""

import numpy as np

import concourse.bass as bass
import concourse.tile as tile
from concourse import bacc
from concourse import mybir

F32 = mybir.dt.float32
BF16 = mybir.dt.bfloat16
I32 = mybir.dt.int32

B, NR, NPTOT, E, F, EIN, H, OUT = 2, 16384, 65536, 262144, 128, 4, 16, 4
EPS = 1e-6
NQ = 4                  # pnode quarters per batch
QP = NPTOT // NQ        # pnodes per core (16384)
G = 104                 # segments per group
EPG = 512               # edge slots per group
NC = EPG // 128         # 128-edge chunks per group (4)
NG = (QP + G - 1) // G  # groups per core (158)
NEP = NG * EPG          # padded edge slots per core
PB = 512                # pnode block width
NPB = QP // PB          # pnode blocks per core (32)
MAGIC = 0x5F3759DF      # fast-inverse-sqrt seed constant

AF = mybir.ActivationFunctionType
ALU = mybir.AluOpType


def _build_nc(skip_bias=False):
    nc = bacc.Bacc("TRN2", target_bir_lowering=False, debug=False)

    def inp(name, shape, dtype=F32):
        return nc.dram_tensor(name, shape, dtype, kind="ExternalInput")

    efT = inp("efT", [EIN, NEP], BF16)
    sfT_d = inp("sfT", [F, NEP], BF16)
    rfT_d = inp("rfT", [F, NEP], BF16)
    oh_d = inp("oh", [128, NG * NC * G], BF16)
    pnT_d = inp("pnT", [F, QP], BF16)
    tau_d = inp("tau", [1, 1])
    m01_d = inp("m01", [1, QP], BF16)

    We1 = inp("We1", [EIN, F], BF16)
    be1 = inp("be1", [F, 1])
    We2 = inp("We2", [F, F], BF16)
    be2 = inp("be2", [1, NC * F], BF16)
    Wu1a = inp("Wu1a", [F, F])
    Wu1b = inp("Wu1b", [F, F], BF16)
    Wu1c = inp("Wu1c", [F, F], BF16)
    bu1 = inp("bu1", [F, 1])
    Wu2 = inp("Wu2", [F, F], BF16)
    bu2 = inp("bu2", [1, NC * F], BF16)
    Wp1n = inp("Wp1n", [F, F], BF16)
    Wp1g = inp("Wp1g", [F, F])
    bp1 = inp("bp1", [F, 1])
    Wp2 = inp("Wp2", [F, F], BF16)
    bp2 = inp("bp2", [1, NC * F], BF16)
    Wo1 = inp("Wo1", [F, F])
    bo1 = inp("bo1", [F, 1])
    Wo2 = inp("Wo2", [F, OUT], BF16)
    bo2 = inp("bo2", [1, OUT], BF16)
    # conditioning nets: e(dge embed), u(pdate), p(node).  r is dead code.
    cond_w = {}
    for k in ("e", "u", "p"):
        cond_w[k] = (
            inp(f"C{k}1", [1, H]),
            inp(f"c{k}1", [H, 1]),
            inp(f"C{k}2a", [H, F]),     # scale half of C2
            inp(f"C{k}2b", [H, F]),     # shift half of C2
            inp(f"c{k}2a1", [F, 1]),    # c2[:F] + 1.0
            inp(f"c{k}2b", [F, 1]),     # c2[F:]
        )

    outT = nc.dram_tensor("outT", [OUT, QP], F32, kind="ExternalOutput")

    from contextlib import ExitStack

    with tile.TileContext(nc) as tc, ExitStack() as ctx:
        singles = ctx.enter_context(tc.tile_pool(name="singles", bufs=1))
        ld = ctx.enter_context(tc.tile_pool(name="ld", bufs=3))
        work = ctx.enter_context(tc.tile_pool(name="work", bufs=4))
        small = ctx.enter_context(tc.tile_pool(name="small", bufs=4))
        psB = ctx.enter_context(tc.tile_pool(name="psB", bufs=2, space="PSUM"))
        psA = ctx.enter_context(tc.tile_pool(name="psA", bufs=4, space="PSUM"))
        psS = ctx.enter_context(tc.tile_pool(name="psS", bufs=2, space="PSUM"))

        # ---------- constants & resident tables ----------
        def load(name, dram, shape, dtype=F32):
            t = singles.tile(shape, dtype, tag=name)
            nc.sync.dma_start(out=t[:], in_=dram[:])
            return t

        sWe1 = load("We1", We1, [EIN, F], BF16)
        sbe1 = load("be1", be1, [F, 1])
        sWe2 = load("We2", We2, [F, F], BF16)
        sbe2 = load("be2", be2, [1, NC * F], BF16)
        sWu1a = load("Wu1a", Wu1a, [F, F])
        sWu1b = load("Wu1b", Wu1b, [F, F], BF16)
        sWu1c = load("Wu1c", Wu1c, [F, F], BF16)
        sbu1 = load("bu1", bu1, [F, 1])
        sWu2 = load("Wu2", Wu2, [F, F], BF16)
        sbu2 = load("bu2", bu2, [1, NC * F], BF16)
        sWp1n = load("Wp1n", Wp1n, [F, F], BF16)
        sWp1g = load("Wp1g", Wp1g, [F, F])
        sbp1 = load("bp1", bp1, [F, 1])
        sWp2 = load("Wp2", Wp2, [F, F], BF16)
        sbp2 = load("bp2", bp2, [1, NC * F], BF16)
        sWo1 = load("Wo1", Wo1, [F, F])
        sbo1 = load("bo1", bo1, [F, 1])
        sWo2 = load("Wo2", Wo2, [F, OUT], BF16)
        sbo2 = load("bo2", bo2, [1, OUT], BF16)
        stau = load("tau", tau_d, [1, 1])
        sm01 = load("m01", m01_d, [1, QP], BF16)

        from concourse.masks import make_identity

        ident = singles.tile([128, 128], F32, tag="ident")
        make_identity(nc, ident[:])
        ident16 = singles.tile([128, 128], BF16, tag="ident16")
        nc.vector.tensor_copy(out=ident16[:], in_=ident[:])
        ones_r = singles.tile([1, PB], BF16, tag="ones_r")
        nc.vector.memset(ones_r[:], 1.0)
        ones_r32 = singles.tile([1, 128], F32, tag="ones_r32")
        nc.vector.memset(ones_r32[:], 1.0)

        # resident aggregate accumulators (segment sums, transposed)
        aggSB1 = singles.tile([128, NG * G], BF16, tag="aggSB1")
        aggSB2 = singles.tile([128, NG * G], BF16, tag="aggSB2")

        # ---------- conditioning nets (tau -> scale/shift) + weight folds ----
        cvec = {}
        for k in ("e", "u", "p"):
            C1, c1, C2a, C2b, c2a1, c2b = cond_w[k]
            sC1 = load(f"C{k}1", C1, [1, H])
            sc1 = load(f"c{k}1", c1, [H, 1])
            sC2a = load(f"C{k}2a", C2a, [H, F])
            sC2b = load(f"C{k}2b", C2b, [H, F])
            sc2a1 = load(f"c{k}2a1", c2a1, [F, 1])
            sc2b = load(f"c{k}2b", c2b, [F, 1])

            ph = psS.tile([H, 1], F32, tag="pS")
            nc.tensor.matmul(ph[:], lhsT=sC1[:], rhs=stau[:], start=True, stop=True)
            hs = small.tile([H, 1], F32, tag=f"hs{k}")
            nc.scalar.activation(hs[:], ph[:], AF.Silu, bias=sc1[:], scale=1.0)

            pscale = psS.tile([F, 1], F32, tag="pS")
            nc.tensor.matmul(pscale[:], lhsT=sC2a[:], rhs=hs[:], start=True, stop=True)
            s1p = singles.tile([F, 1], F32, tag=f"s1p{k}")
            nc.vector.tensor_scalar(
                out=s1p[:], in0=pscale[:], scalar1=sc2a1[:], scalar2=None, op0=ALU.add
            )
            pshift = psS.tile([F, 1], F32, tag="pS")
            nc.tensor.matmul(pshift[:], lhsT=sC2b[:], rhs=hs[:], start=True, stop=True)
            shift = singles.tile([F, 1], F32, tag=f"shift{k}")
            nc.vector.tensor_scalar(
                out=shift[:], in0=pshift[:], scalar1=sc2b[:], scalar2=None, op0=ALU.add
            )
            cvec[k] = (s1p, shift)

        s1pe, shifte = cvec["e"]
        s1pu, shiftu = cvec["u"]
        s1pp, shiftp = cvec["p"]

        # fold cond-norm affines into downstream weights
        fWu1a = singles.tile([F, F], BF16, tag="fWu1a")
        nc.vector.tensor_tensor(
            out=fWu1a[:], in0=sWu1a[:], in1=s1pe[:].to_broadcast([F, F]), op=ALU.mult
        )
        pbu1 = psS.tile([F, 1], F32, tag="pS")
        nc.tensor.matmul(pbu1[:], lhsT=sWu1a[:], rhs=shifte[:], start=True, stop=True)
        fbu1 = singles.tile([F, 1], F32, tag="fbu1")
        nc.vector.tensor_scalar(
            out=fbu1[:], in0=pbu1[:], scalar1=sbu1[:], scalar2=None, op0=ALU.add
        )

        fWp1ge = singles.tile([F, F], BF16, tag="fWp1ge")
        nc.vector.tensor_tensor(
            out=fWp1ge[:], in0=sWp1g[:], in1=s1pe[:].to_broadcast([F, F]), op=ALU.mult
        )
        fWp1gu = singles.tile([F, F], BF16, tag="fWp1gu")
        nc.vector.tensor_tensor(
            out=fWp1gu[:], in0=sWp1g[:], in1=s1pu[:].to_broadcast([F, F]), op=ALU.mult
        )
        shifteu = small.tile([F, 1], F32, tag="shifteu")
        nc.vector.tensor_tensor(
            out=shifteu[:], in0=shifte[:], in1=shiftu[:], op=ALU.add
        )
        pbpe = psS.tile([1, F], F32, tag="pS")
        nc.tensor.matmul(pbpe[:], lhsT=shifteu[:], rhs=sWp1g[:], start=True, stop=True)
        bpe_row = singles.tile([1, F], BF16, tag="bpe_row")
        nc.vector.tensor_copy(out=bpe_row[:], in_=pbpe[:])

        fWo1 = singles.tile([F, F], BF16, tag="fWo1")
        nc.vector.tensor_tensor(
            out=fWo1[:], in0=sWo1[:], in1=s1pp[:].to_broadcast([F, F]), op=ALU.mult
        )
        sWo116 = singles.tile([F, F], BF16, tag="sWo116")
        nc.vector.tensor_copy(out=sWo116[:], in_=sWo1[:])
        pbo1 = psS.tile([F, 1], F32, tag="pS")
        nc.tensor.matmul(pbo1[:], lhsT=sWo1[:], rhs=shiftp[:], start=True, stop=True)
        fbo1 = singles.tile([F, 1], F32, tag="fbo1")
        nc.vector.tensor_scalar(
            out=fbo1[:], in0=pbo1[:], scalar1=sbo1[:], scalar2=None, op0=ALU.add
        )

        # LN bn-stats into a shared mv view (cols of a [128, 8, 2] tile)
        def ln_bn(psum4, nch, mv_view, tag):
            for c in range(nch):
                st6 = small.tile([128, 6], F32, tag=f"st{tag}", name="st6")
                nc.vector.bn_stats(out=st6[:], in_=psum4[:, c, :])
                nc.vector.bn_aggr(out=mv_view[:, c, :], in_=st6[:])

        # combined fast-inverse-sqrt over a [128, n, 2] mv tile -> rstd/negms
        def fisr_n(mvc, n, tag, iters=1):
            a = small.tile([128, n], F32, tag=f"a{tag}", name="a")
            nc.vector.tensor_scalar(
                out=a[:], in0=mvc[:, :, 1], scalar1=EPS, scalar2=None, op0=ALU.add
            )
            bi = small.tile([128, n], I32, tag=f"bi{tag}", name="bi")
            nc.vector.tensor_scalar(
                out=bi[:], in0=a[:].bitcast(I32), scalar1=1, scalar2=None,
                op0=ALU.arith_shift_right,
            )
            y0 = small.tile([128, n], F32, tag=f"y0{tag}", name="y0")
            nc.vector.tensor_scalar(
                out=y0[:].bitcast(I32), in0=bi[:], scalar1=-1, scalar2=MAGIC,
                op0=ALU.mult, op1=ALU.add,
            )
            ha = small.tile([128, n], F32, tag=f"ha{tag}", name="ha")
            nc.vector.tensor_scalar(
                out=ha[:], in0=a[:], scalar1=-0.5, scalar2=None, op0=ALU.mult
            )
            y = y0
            for it in range(iters):
                yy = small.tile([128, n], F32, tag=f"yy{tag}{it}", name="yy")
                nc.vector.tensor_tensor(out=yy[:], in0=y[:], in1=y[:], op=ALU.mult)
                hyy = small.tile([128, n], F32, tag=f"hy{tag}{it}", name="hyy")
                nc.vector.tensor_tensor(out=hyy[:], in0=yy[:], in1=ha[:], op=ALU.mult)
                yn = small.tile([128, n], F32, tag=f"yn{tag}{it}", name="yn")
                nc.vector.scalar_tensor_tensor(
                    out=yn[:], in0=hyy[:], scalar=1.5, in1=y[:],
                    op0=ALU.add, op1=ALU.mult,
                )
                y = yn
            negms = small.tile([128, n], F32, tag=f"nm{tag}", name="negms")
            nc.vector.scalar_tensor_tensor(
                out=negms[:], in0=mvc[:, :, 0], scalar=-1.0, in1=y[:],
                op0=ALU.mult, op1=ALU.mult,
            )
            return y, negms

        # LayerNorm stats helper: psum4 [128, nch, 128] -> (mv, rstd, negms)
        #   mv[:, c, 0] = mean, rstd = 1/sqrt(var+eps), negms = -mean*rstd
        # bn runs on vector (PSUM reads); the rsqrt Newton chain runs on the
        # otherwise-idle gpsimd engine (SBUF-only tiles).
        def ln_stats(psum4, nch, tag, iters=1):
            mv = small.tile([128, nch, 2], F32, tag=f"mv{tag}")
            for c in range(nch):
                st6 = small.tile([128, 6], F32, tag=f"st{tag}")
                nc.vector.bn_stats(out=st6[:], in_=psum4[:, c, :])
                nc.vector.bn_aggr(out=mv[:, c, :], in_=st6[:])
            a = small.tile([128, nch], F32, tag=f"a{tag}")
            nc.vector.tensor_scalar(
                out=a[:], in0=mv[:, :, 1], scalar1=EPS, scalar2=None, op0=ALU.add
            )
            bi = small.tile([128, nch], I32, tag=f"bi{tag}")
            nc.vector.tensor_scalar(
                out=bi[:], in0=a[:].bitcast(I32), scalar1=1, scalar2=None,
                op0=ALU.arith_shift_right,
            )
            y0 = small.tile([128, nch], F32, tag=f"y0{tag}")
            nc.vector.tensor_scalar(
                out=y0[:].bitcast(I32), in0=bi[:], scalar1=-1, scalar2=MAGIC,
                op0=ALU.mult, op1=ALU.add,
            )
            ha = small.tile([128, nch], F32, tag=f"ha{tag}")
            nc.vector.tensor_scalar(
                out=ha[:], in0=a[:], scalar1=-0.5, scalar2=None, op0=ALU.mult
            )
            y = y0
            for it in range(iters):
                yy = small.tile([128, nch], F32, tag=f"yy{tag}{it}")
                nc.vector.tensor_tensor(out=yy[:], in0=y[:], in1=y[:], op=ALU.mult)
                hyy = small.tile([128, nch], F32, tag=f"hy{tag}{it}")
                nc.vector.tensor_tensor(out=hyy[:], in0=yy[:], in1=ha[:], op=ALU.mult)
                yn = small.tile([128, nch], F32, tag=f"yn{tag}{it}")
                nc.vector.scalar_tensor_tensor(
                    out=yn[:], in0=hyy[:], scalar=1.5, in1=y[:],
                    op0=ALU.add, op1=ALU.mult,
                )
                y = yn
            negms = small.tile([128, nch], F32, tag=f"nm{tag}")
            nc.vector.scalar_tensor_tensor(
                out=negms[:], in0=mv[:, :, 0], scalar=-1.0, in1=y[:],
                op0=ALU.mult, op1=ALU.mult,
            )
            return mv, y, negms

        # ---------- edge phase (4-stage software pipeline over groups) ------
        # S0: DMA loads   S1: embed MLP + LN-e stats   S2: apply-e, transpose,
        # update MLP + LN-u stats   S3: apply-u, one-hot aggregation.
        # Stages of group g are issued in different loop iterations so every
        # engine's program interleaves adjacent groups (no head-of-line
        # blocking on cross-engine dependencies).

        def edge_s0(g):
            # loads groups g and g+1 in one DMA per stream (g is even)
            hi = min(g + 2, NG)
            w = hi - g
            esl = slice(g * EPG, hi * EPG)
            t = {}
            t["efg"] = ld.tile([EIN, 2 * EPG], BF16, tag="efg", name="efg")
            nc.sync.dma_start(out=t["efg"][:, : w * EPG], in_=efT[:, esl])
            t["sfg"] = ld.tile([128, 2 * EPG], BF16, tag="sfg", name="sfg")
            nc.sync.dma_start(out=t["sfg"][:, : w * EPG], in_=sfT_d[:, esl])
            t["rfg"] = ld.tile([128, 2 * EPG], BF16, tag="rfg", name="rfg")
            nc.sync.dma_start(out=t["rfg"][:, : w * EPG], in_=rfT_d[:, esl])
            t["ohg"] = ld.tile([128, 2 * NC, G], BF16, tag="ohg", name="ohg")
            nc.sync.dma_start(
                out=t["ohg"][:, : w * NC, :],
                in_=oh_d[:, g * NC * G : hi * NC * G],
            )
            return t

        def edge_s1(g, t):
            h = g % 2
            pz1 = psB.tile([128, EPG], F32, tag="pB")
            nc.tensor.matmul(
                pz1[:], lhsT=sWe1[:],
                rhs=t["efg"][:, h * EPG : (h + 1) * EPG], start=True, stop=True,
            )
            y1 = work.tile([128, EPG], BF16, tag="y1")
            nc.scalar.activation(y1[:], pz1[:], AF.Silu, bias=sbe1[:], scale=1.0)

            pz2 = psA.tile([128, NC, 128], F32, tag="pA")
            if not skip_bias:
                nc.tensor.matmul(
                    pz2[:, :, :], lhsT=ones_r[:, :128], rhs=sbe2[:],
                    start=True, stop=False, skip_group_check=True,
                )
            for c in range(NC):
                nc.tensor.matmul(
                    pz2[:, c, :],
                    lhsT=y1[:, c * 128 : (c + 1) * 128],
                    rhs=sWe2[:],
                    start=skip_bias,
                    stop=skip_bias or (c == NC - 1),
                    skip_group_check=True,
                )
            t["pz2"] = pz2
            mve = small.tile([128, 4, 2], F32, tag="mve", name="mve")
            ln_bn(pz2, NC, mve[:, :, :], "e")
            t["mve"] = mve
            t["fe"] = fisr_n(mve, 4, "e")

        def edge_s2(g, t):
            pz2 = t["pz2"]
            rs8, nm8 = t["fe"]
            ln1 = work.tile([128, NC, 128], BF16, tag="ln1")
            for c in range(NC):
                nc.scalar.activation(
                    ln1[:, c, :], pz2[:, c, :], AF.Identity,
                    bias=nm8[:, c : c + 1], scale=rs8[:, c : c + 1],
                )
            t["ln1"] = ln1

            ptr = psS.tile([128, NC, 128], BF16, tag="pS")
            for c in range(NC):
                nc.tensor.transpose(ptr[:, c, :], ln1[:, c, :], ident16[:])
            ln1T = work.tile([128, NC // 2, 2, 128], BF16, tag="ln1T")
            nc.vector.tensor_copy(out=ln1T[:, :, 0, :], in_=ptr[:, 0::2, :])
            nc.scalar.activation(ln1T[:, :, 1, :], ptr[:, 1::2, :], AF.Copy)

            pu1 = psB.tile([128, EPG], F32, tag="pB")
            nc.tensor.matmul(
                pu1[:], lhsT=fWu1a[:], rhs=ln1T[:],
                start=True, stop=False,
            )
            h = g % 2
            nc.tensor.matmul(
                pu1[:], lhsT=sWu1b[:],
                rhs=t["sfg"][:, h * EPG : (h + 1) * EPG],
                start=False, stop=False,
            )
            nc.tensor.matmul(
                pu1[:], lhsT=sWu1c[:],
                rhs=t["rfg"][:, h * EPG : (h + 1) * EPG],
                start=False, stop=True,
            )
            yu = work.tile([128, EPG], BF16, tag="yu")
            nc.scalar.activation(yu[:], pu1[:], AF.Silu, bias=fbu1[:], scale=1.0)

            pu2 = psA.tile([128, NC, 128], F32, tag="pA")
            if not skip_bias:
                nc.tensor.matmul(
                    pu2[:, :, :], lhsT=ones_r[:, :128], rhs=sbu2[:],
                    start=True, stop=False, skip_group_check=True,
                )
            for c in range(NC):
                nc.tensor.matmul(
                    pu2[:, c, :],
                    lhsT=yu[:, c * 128 : (c + 1) * 128],
                    rhs=sWu2[:],
                    start=skip_bias,
                    stop=skip_bias or (c == NC - 1),
                    skip_group_check=True,
                )
            t["pu2"] = pu2
            mvu = small.tile([128, 4, 2], F32, tag="mvu", name="mvu")
            ln_bn(pu2, NC, mvu[:, :, :], "u")
            t["mvu"] = mvu
            t["fu"] = fisr_n(mvu, 4, "u")

        def edge_s3(g, t):
            pu2 = t["pu2"]
            rs8, nm8 = t["fu"]
            mvu = t["mvu"]
            ln1 = t["ln1"]
            ln2 = work.tile([128, NC, 128], BF16, tag="ln2")
            for c in range(NC):
                if c % 2 == 0:
                    nc.scalar.activation(
                        ln2[:, c, :], pu2[:, c, :], AF.Identity,
                        bias=nm8[:, c : c + 1], scale=rs8[:, c : c + 1],
                    )
                else:
                    nc.vector.tensor_scalar(
                        out=ln2[:, c, :], in0=pu2[:, c, :],
                        scalar1=mvu[:, c, 0:1], scalar2=rs8[:, c : c + 1],
                        op0=ALU.subtract, op1=ALU.mult,
                    )

            h = g % 2
            Sps = psS.tile([128, 2 * G], F32, tag="pS")
            for c in range(NC):
                nc.tensor.matmul(
                    Sps[:, 0:G], lhsT=ln1[:, c, :],
                    rhs=t["ohg"][:, h * NC + c, :],
                    start=(c == 0), stop=(c == NC - 1),
                )
            for c in range(NC):
                nc.tensor.matmul(
                    Sps[:, G : 2 * G], lhsT=ln2[:, c, :],
                    rhs=t["ohg"][:, h * NC + c, :],
                    start=(c == 0), stop=(c == NC - 1),
                )
            nc.vector.tensor_copy(
                out=aggSB1[:, g * G : (g + 1) * G], in_=Sps[:, 0:G]
            )
            nc.vector.tensor_copy(
                out=aggSB2[:, g * G : (g + 1) * G], in_=Sps[:, G : 2 * G]
            )

        state = {}
        shared = {}
        for i in range(NG + 3):
            if i < NG and i % 2 == 0:
                shared[i] = edge_s0(i)
                state[i] = dict(shared[i])
                if i + 1 < NG:
                    state[i + 1] = dict(shared[i])
            if 0 <= i - 1 < NG:
                edge_s1(i - 1, state[i - 1])
            if 0 <= i - 2 < NG:
                edge_s2(i - 2, state[i - 2])
            if 0 <= i - 3 < NG:
                edge_s3(i - 3, state[i - 3])
                del state[i - 3]

        # ---------- pnode phase (2-stage software pipeline over blocks) -----
        def pn_s1(j):
            sl = slice(j * PB, (j + 1) * PB)
            t = {"sl": sl}
            pn16b = ld.tile([128, PB], BF16, tag="pn16b")
            nc.sync.dma_start(out=pn16b[:], in_=pnT_d[:, sl])
            t["pn16b"] = pn16b

            pzp = psB.tile([128, PB], F32, tag="pB")
            nc.tensor.matmul(pzp[:], lhsT=sWp1n[:], rhs=pn16b[:], start=True, stop=False)
            nc.tensor.matmul(
                pzp[:], lhsT=fWp1ge[:], rhs=aggSB1[:, sl], start=False, stop=False
            )
            nc.tensor.matmul(
                pzp[:], lhsT=fWp1gu[:], rhs=aggSB2[:, sl], start=False, stop=False
            )
            nc.tensor.matmul(
                pzp[:], lhsT=bpe_row[:], rhs=sm01[:, sl], start=False, stop=True
            )
            yp = work.tile([128, PB], BF16, tag="yu")
            nc.scalar.activation(yp[:], pzp[:], AF.Silu, bias=sbp1[:], scale=1.0)

            pp2 = psA.tile([128, NC, 128], F32, tag="pA")
            if not skip_bias:
                nc.tensor.matmul(
                    pp2[:, :, :], lhsT=ones_r[:, :128], rhs=sbp2[:],
                    start=True, stop=False, skip_group_check=True,
                )
            for c in range(NC):
                nc.tensor.matmul(
                    pp2[:, c, :],
                    lhsT=yp[:, c * 128 : (c + 1) * 128],
                    rhs=sWp2[:],
                    start=skip_bias,
                    stop=skip_bias or (c == NC - 1),
                    skip_group_check=True,
                )
            t["pp2"] = pp2
            mvp = small.tile([128, 4, 2], F32, tag="mvp", name="mvp")
            ln_bn(pp2, NC, mvp[:, :, :], "p")
            t["mvp"] = mvp
            t["fp"] = fisr_n(mvp, 4, "p")
            return t

        def pn_s2(j, t):
            sl = t["sl"]
            pp2 = t["pp2"]
            rsp, nmp = t["fp"]
            lnp = work.tile([128, NC, 128], BF16, tag="ln1")
            for c in range(NC):
                nc.scalar.activation(
                    lnp[:, c, :], pp2[:, c, :], AF.Identity,
                    bias=nmp[:, c : c + 1], scale=rsp[:, c : c + 1],
                )

            ptr2 = psS.tile([128, NC, 128], BF16, tag="pS")
            for c in range(NC):
                nc.tensor.transpose(ptr2[:, c, :], lnp[:, c, :], ident16[:])
            lnpT = work.tile([128, NC // 2, 2, 128], BF16, tag="ln1T")
            nc.vector.tensor_copy(out=lnpT[:, :, 0, :], in_=ptr2[:, 0::2, :])
            nc.scalar.activation(lnpT[:, :, 1, :], ptr2[:, 1::2, :], AF.Copy)

            pzo = psB.tile([128, PB], F32, tag="pB")
            nc.tensor.matmul(
                pzo[:], lhsT=fWo1[:], rhs=lnpT[:],
                start=True, stop=False,
            )
            nc.tensor.matmul(
                pzo[:], lhsT=sWo116[:], rhs=t["pn16b"][:], start=False, stop=True
            )
            yo = work.tile([128, PB], BF16, tag="ln2")
            nc.scalar.activation(yo[:], pzo[:], AF.Silu, bias=fbo1[:], scale=1.0)

            po = psS.tile([OUT, PB], F32, tag="pS")
            nc.tensor.matmul(po[:], lhsT=sWo2[:], rhs=yo[:], start=True, stop=False)
            nc.tensor.matmul(po[:], lhsT=sbo2[:], rhs=ones_r[:], start=False, stop=True)
            oc = work.tile([OUT, PB], F32, tag="oc")
            nc.vector.tensor_copy(out=oc[:], in_=po[:])
            nc.sync.dma_start(out=outT[:, sl], in_=oc[:])

        pstate = {}
        for i in range(NPB + 1):
            if i < NPB:
                pstate[i] = pn_s1(i)
            if 0 <= i - 1 < NPB:
                pn_s2(i - 1, pstate[i - 1])
                del pstate[i - 1]

    nc.compile()
    return nc


def _prep_core(ef_b, snd_b, rcv_b, rn_b, pn_b, tau_b, q):
    import ml_dtypes

    lo = q * QP
    mask = (rcv_b >= lo) & (rcv_b < lo + QP)
    ed = np.nonzero(mask)[0]
    loc = (rcv_b[ed] - lo).astype(np.int64)
    order = np.argsort(loc, kind="stable")
    ed, loc = ed[order], loc[order]
    grp = loc // G
    cnts = np.bincount(grp, minlength=NG)
    assert cnts.max() <= EPG, f"group overflow: {cnts.max()} > {EPG}"
    gstart = np.concatenate([[0], np.cumsum(cnts)[:-1]])
    slot = grp * EPG + (np.arange(len(ed)) - gstart[grp])

    efp = np.zeros((NEP, EIN), np.float32)
    efp[slot] = ef_b[ed]
    sf = np.zeros((NEP, F), np.float32)
    sf[slot] = rn_b[snd_b[ed]]
    rf = np.zeros((NEP, F), np.float32)
    rf[slot] = pn_b[lo + rcv_b[ed] - lo]
    cnt_all = np.bincount(loc, minlength=QP).astype(np.float32)
    ohf = np.zeros((NEP, G), np.float32)
    ohf[slot, loc - grp * G] = 1.0 / cnt_all[loc]
    oh_dev = np.ascontiguousarray(
        ohf.reshape(NG, NC, 128, G).transpose(2, 0, 1, 3).reshape(128, NG * NC * G)
    )

    m01_seg = np.minimum(cnt_all, 1.0)

    pn_q = pn_b[lo : lo + QP]
    bf = ml_dtypes.bfloat16
    return {
        "m01": m01_seg.reshape(1, QP).astype(bf),
        "efT": np.ascontiguousarray(efp.T.astype(bf)),
        "sfT": np.ascontiguousarray(sf.T.astype(bf)),
        "rfT": np.ascontiguousarray(rf.T.astype(bf)),
        "oh": oh_dev.astype(bf),
        "pnT": np.ascontiguousarray(pn_q.T.astype(bf)),
        "tau": tau_b.reshape(1, 1).astype(np.float32),
    }


def _prep_weights(i):
    w = {
        "We1": i["We1"], "be1": i["be1"].reshape(F, 1), "We2": i["We2"],
        "be2": np.tile(i["be2"].reshape(1, F), (1, NC)),
        "Wu1a": i["Wu1"][0:F], "Wu1b": i["Wu1"][F : 2 * F],
        "Wu1c": i["Wu1"][2 * F : 3 * F],
        "bu1": i["bu1"].reshape(F, 1), "Wu2": i["Wu2"],
        "bu2": np.tile(i["bu2"].reshape(1, F), (1, NC)),
        "Wp1n": i["Wp1"][0:F], "Wp1g": i["Wp1"][F : 2 * F],
        "bp1": i["bp1"].reshape(F, 1), "Wp2": i["Wp2"],
        "bp2": np.tile(i["bp2"].reshape(1, F), (1, NC)),
        "Wo1": i["Wo1"], "bo1": i["bo1"].reshape(F, 1), "Wo2": i["Wo2"],
        "bo2": i["bo2"].reshape(1, OUT),
    }
    for k in ("e", "u", "p"):
        C1, c1 = i[f"C{k}1"], i[f"c{k}1"]
        C2, c2 = i[f"C{k}2"], i[f"c{k}2"]
        w[f"C{k}1"] = C1.reshape(1, H)
        w[f"c{k}1"] = c1.reshape(H, 1)
        w[f"C{k}2a"] = np.ascontiguousarray(C2[:, 0:F])
        w[f"C{k}2b"] = np.ascontiguousarray(C2[:, F : 2 * F])
        w[f"c{k}2a1"] = (c2[0:F] + 1.0).reshape(F, 1)
        w[f"c{k}2b"] = c2[F : 2 * F].reshape(F, 1)
    import ml_dtypes

    bf16_keys = {"We1", "We2", "Wu1b", "Wu1c", "Wu2", "Wp1n", "Wp2", "Wo2",
                 "be2", "bu2", "bp2", "bo2"}
    return {
        k: np.ascontiguousarray(
            v, dtype=ml_dtypes.bfloat16 if k in bf16_keys else np.float32
        )
        for k, v in w.items()
    }


_NC_CACHE = {}


def _all_bias_zero(i):
    return all(
        not np.any(np.asarray(i[k]))
        for k in ("be2", "bu2", "bp2")
    )


def build_in_maps(inputs):
    i = {k: np.asarray(v) for k, v in inputs.items()}
    w = _prep_weights(i)
    in_maps = []
    for core in range(8):
        b, q = core // NQ, core % NQ
        m = dict(w)
        m.update(
            _prep_core(
                i["edge_features"][b], i["senders"][b], i["receivers"][b],
                i["rnode_features"][b], i["pnode_features"][b], i["tau"][b], q
            )
        )
        in_maps.append(m)
    return in_maps


def get_nc(skip_bias=False):
    key = ("nc", bool(skip_bias))
    if key not in _NC_CACHE:
        _NC_CACHE[key] = _build_nc(skip_bias=skip_bias)
    return _NC_CACHE[key]


def assemble(results):
    out = np.zeros((B, NPTOT, OUT), np.float32)
    for core in range(8):
        b, q = core // NQ, core % NQ
        out[b, q * QP : (q + 1) * QP, :] = results[core]["outT"].T
    return out


def kernel(**inputs):
    from concourse.bass_utils import run_bass_kernel_spmd

    nc = get_nc(skip_bias=_all_bias_zero(inputs))
    in_maps = build_in_maps(inputs)
    res = run_bass_kernel_spmd(nc, in_maps, list(range(8)))
    return assemble(res.results)


if __name__ == "__main__":
    import reference

    inputs = reference.setup_inputs()
    out = kernel(**{k: np.asarray(v) for k, v in inputs.items()})
    print("out", out.shape, out.dtype)


# revision 31
# speedup vs baseline: 1.4628x; 1.0433x over previous
"""Trainium2 Bass kernel for the GNN decoder (message passing, cond-layernorm).

Sharding: 8 cores = (batch b in {0,1}) x (pnode quarter q in {0..3}).
Each core owns pnode rows [q*16384, (q+1)*16384) of its batch and every edge
whose receiver lands in that range.  Edges are receiver-sorted on the host and
packed into NG groups of G=104 consecutive segments with a fixed budget of
EPG=512 edge slots per group (padded; pad slots have a zero one-hot row so
they contribute nothing).  The host also pre-gathers sender/receiver features
per edge slot (transposed, bf16) and pre-builds the segment one-hots, so the
device runs a pure dense pipeline: embed MLP -> cond LN -> update MLP ->
cond LN -> one-hot segment-sum matmuls, with the aggregate kept resident in
SBUF for the pnode phase.  LayerNorm rstd uses a fast-inverse-sqrt (bitcast +
Newton) on the vector engine so the scalar engine never leaves the silu
activation-table set.  Cond-norm output affines (1+scale, shift) are folded
into the next matmul's weights on device once per launch.
"""

import numpy as np

import concourse.bass as bass
import concourse.tile as tile
from concourse import bacc
from concourse import mybir

F32 = mybir.dt.float32
BF16 = mybir.dt.bfloat16
I32 = mybir.dt.int32

B, NR, NPTOT, E, F, EIN, H, OUT = 2, 16384, 65536, 262144, 128, 4, 16, 4
EPS = 1e-6
NQ = 4                  # pnode quarters per batch
QP = NPTOT // NQ        # pnodes per core (16384)
G = 104                 # segments per group
EPG = 512               # edge slots per group
NC = EPG // 128         # 128-edge chunks per group (4)
NG = (QP + G - 1) // G  # groups per core (158)
NEP = NG * EPG          # padded edge slots per core
PB = 512                # pnode block width
NPB = QP // PB          # pnode blocks per core (32)
MAGIC = 0x5F3759DF      # fast-inverse-sqrt seed constant

AF = mybir.ActivationFunctionType
ALU = mybir.AluOpType


def _build_nc(skip_bias=False):
    nc = bacc.Bacc("TRN2", target_bir_lowering=False, debug=False)

    def inp(name, shape, dtype=F32):
        return nc.dram_tensor(name, shape, dtype, kind="ExternalInput")

    efT = inp("efT", [EIN, NEP], BF16)
    sfT_d = inp("sfT", [F, NEP], BF16)
    rfT_d = inp("rfT", [F, NEP], BF16)
    oh_d = inp("oh", [128, NG * NC * G], BF16)
    pnT_d = inp("pnT", [F, QP], BF16)
    tau_d = inp("tau", [1, 1])
    m01_d = inp("m01", [1, QP], BF16)

    We1 = inp("We1", [EIN, F], BF16)
    be1 = inp("be1", [F, 1])
    We2 = inp("We2", [F, F], BF16)
    be2 = inp("be2", [1, NC * F], BF16)
    Wu1a = inp("Wu1a", [F, F])
    Wu1b = inp("Wu1b", [F, F], BF16)
    Wu1c = inp("Wu1c", [F, F], BF16)
    bu1 = inp("bu1", [F, 1])
    Wu2 = inp("Wu2", [F, F], BF16)
    bu2 = inp("bu2", [1, NC * F], BF16)
    Wp1n = inp("Wp1n", [F, F], BF16)
    Wp1g = inp("Wp1g", [F, F])
    bp1 = inp("bp1", [F, 1])
    Wp2 = inp("Wp2", [F, F], BF16)
    bp2 = inp("bp2", [1, NC * F], BF16)
    Wo1 = inp("Wo1", [F, F])
    bo1 = inp("bo1", [F, 1])
    Wo2 = inp("Wo2", [F, OUT], BF16)
    bo2 = inp("bo2", [1, OUT], BF16)
    # conditioning nets: e(dge embed), u(pdate), p(node).  r is dead code.
    cond_w = {}
    for k in ("e", "u", "p"):
        cond_w[k] = (
            inp(f"C{k}1", [1, H]),
            inp(f"c{k}1", [H, 1]),
            inp(f"C{k}2a", [H, F]),     # scale half of C2
            inp(f"C{k}2b", [H, F]),     # shift half of C2
            inp(f"c{k}2a1", [F, 1]),    # c2[:F] + 1.0
            inp(f"c{k}2b", [F, 1]),     # c2[F:]
        )

    outT = nc.dram_tensor("outT", [OUT, QP], F32, kind="ExternalOutput")

    from contextlib import ExitStack

    with tile.TileContext(nc) as tc, ExitStack() as ctx:
        singles = ctx.enter_context(tc.tile_pool(name="singles", bufs=1))
        ld = ctx.enter_context(tc.tile_pool(name="ld", bufs=3))
        work = ctx.enter_context(tc.tile_pool(name="work", bufs=4))
        small = ctx.enter_context(tc.tile_pool(name="small", bufs=4))
        psB = ctx.enter_context(tc.tile_pool(name="psB", bufs=2, space="PSUM"))
        psA = ctx.enter_context(tc.tile_pool(name="psA", bufs=4, space="PSUM"))
        psS = ctx.enter_context(tc.tile_pool(name="psS", bufs=2, space="PSUM"))

        # ---------- constants & resident tables ----------
        def load(name, dram, shape, dtype=F32):
            t = singles.tile(shape, dtype, tag=name)
            nc.sync.dma_start(out=t[:], in_=dram[:])
            return t

        sWe1 = load("We1", We1, [EIN, F], BF16)
        sbe1 = load("be1", be1, [F, 1])
        sWe2 = load("We2", We2, [F, F], BF16)
        sbe2 = load("be2", be2, [1, NC * F], BF16)
        sWu1a = load("Wu1a", Wu1a, [F, F])
        sWu1b = load("Wu1b", Wu1b, [F, F], BF16)
        sWu1c = load("Wu1c", Wu1c, [F, F], BF16)
        sbu1 = load("bu1", bu1, [F, 1])
        sWu2 = load("Wu2", Wu2, [F, F], BF16)
        sbu2 = load("bu2", bu2, [1, NC * F], BF16)
        sWp1n = load("Wp1n", Wp1n, [F, F], BF16)
        sWp1g = load("Wp1g", Wp1g, [F, F])
        sbp1 = load("bp1", bp1, [F, 1])
        sWp2 = load("Wp2", Wp2, [F, F], BF16)
        sbp2 = load("bp2", bp2, [1, NC * F], BF16)
        sWo1 = load("Wo1", Wo1, [F, F])
        sbo1 = load("bo1", bo1, [F, 1])
        sWo2 = load("Wo2", Wo2, [F, OUT], BF16)
        sbo2 = load("bo2", bo2, [1, OUT], BF16)
        stau = load("tau", tau_d, [1, 1])
        sm01 = load("m01", m01_d, [1, QP], BF16)

        from concourse.masks import make_identity

        ident = singles.tile([128, 128], F32, tag="ident")
        make_identity(nc, ident[:])
        ident16 = singles.tile([128, 128], BF16, tag="ident16")
        nc.vector.tensor_copy(out=ident16[:], in_=ident[:])
        ones_r = singles.tile([1, PB], BF16, tag="ones_r")
        nc.vector.memset(ones_r[:], 1.0)
        ones_r32 = singles.tile([1, 128], F32, tag="ones_r32")
        nc.vector.memset(ones_r32[:], 1.0)

        # resident aggregate accumulators (segment sums, transposed)
        aggSB1 = singles.tile([128, NG * G], BF16, tag="aggSB1")
        aggSB2 = singles.tile([128, NG * G], BF16, tag="aggSB2")

        # ---------- conditioning nets (tau -> scale/shift) + weight folds ----
        cvec = {}
        for k in ("e", "u", "p"):
            C1, c1, C2a, C2b, c2a1, c2b = cond_w[k]
            sC1 = load(f"C{k}1", C1, [1, H])
            sc1 = load(f"c{k}1", c1, [H, 1])
            sC2a = load(f"C{k}2a", C2a, [H, F])
            sC2b = load(f"C{k}2b", C2b, [H, F])
            sc2a1 = load(f"c{k}2a1", c2a1, [F, 1])
            sc2b = load(f"c{k}2b", c2b, [F, 1])

            ph = psS.tile([H, 1], F32, tag="pS")
            nc.tensor.matmul(ph[:], lhsT=sC1[:], rhs=stau[:], start=True, stop=True)
            hs = small.tile([H, 1], F32, tag=f"hs{k}")
            nc.scalar.activation(hs[:], ph[:], AF.Silu, bias=sc1[:], scale=1.0)

            pscale = psS.tile([F, 1], F32, tag="pS")
            nc.tensor.matmul(pscale[:], lhsT=sC2a[:], rhs=hs[:], start=True, stop=True)
            s1p = singles.tile([F, 1], F32, tag=f"s1p{k}")
            nc.vector.tensor_scalar(
                out=s1p[:], in0=pscale[:], scalar1=sc2a1[:], scalar2=None, op0=ALU.add
            )
            pshift = psS.tile([F, 1], F32, tag="pS")
            nc.tensor.matmul(pshift[:], lhsT=sC2b[:], rhs=hs[:], start=True, stop=True)
            shift = singles.tile([F, 1], F32, tag=f"shift{k}")
            nc.vector.tensor_scalar(
                out=shift[:], in0=pshift[:], scalar1=sc2b[:], scalar2=None, op0=ALU.add
            )
            cvec[k] = (s1p, shift)

        s1pe, shifte = cvec["e"]
        s1pu, shiftu = cvec["u"]
        s1pp, shiftp = cvec["p"]

        # fold cond-norm affines into downstream weights
        fWu1a = singles.tile([F, F], BF16, tag="fWu1a")
        nc.vector.tensor_tensor(
            out=fWu1a[:], in0=sWu1a[:], in1=s1pe[:].to_broadcast([F, F]), op=ALU.mult
        )
        pbu1 = psS.tile([F, 1], F32, tag="pS")
        nc.tensor.matmul(pbu1[:], lhsT=sWu1a[:], rhs=shifte[:], start=True, stop=True)
        fbu1 = singles.tile([F, 1], F32, tag="fbu1")
        nc.vector.tensor_scalar(
            out=fbu1[:], in0=pbu1[:], scalar1=sbu1[:], scalar2=None, op0=ALU.add
        )

        fWp1ge = singles.tile([F, F], BF16, tag="fWp1ge")
        nc.vector.tensor_tensor(
            out=fWp1ge[:], in0=sWp1g[:], in1=s1pe[:].to_broadcast([F, F]), op=ALU.mult
        )
        fWp1gu = singles.tile([F, F], BF16, tag="fWp1gu")
        nc.vector.tensor_tensor(
            out=fWp1gu[:], in0=sWp1g[:], in1=s1pu[:].to_broadcast([F, F]), op=ALU.mult
        )
        shifteu = small.tile([F, 1], F32, tag="shifteu")
        nc.vector.tensor_tensor(
            out=shifteu[:], in0=shifte[:], in1=shiftu[:], op=ALU.add
        )
        pbpe = psS.tile([1, F], F32, tag="pS")
        nc.tensor.matmul(pbpe[:], lhsT=shifteu[:], rhs=sWp1g[:], start=True, stop=True)
        bpe_row = singles.tile([1, F], BF16, tag="bpe_row")
        nc.vector.tensor_copy(out=bpe_row[:], in_=pbpe[:])

        fWo1 = singles.tile([F, F], BF16, tag="fWo1")
        nc.vector.tensor_tensor(
            out=fWo1[:], in0=sWo1[:], in1=s1pp[:].to_broadcast([F, F]), op=ALU.mult
        )
        sWo116 = singles.tile([F, F], BF16, tag="sWo116")
        nc.vector.tensor_copy(out=sWo116[:], in_=sWo1[:])
        pbo1 = psS.tile([F, 1], F32, tag="pS")
        nc.tensor.matmul(pbo1[:], lhsT=sWo1[:], rhs=shiftp[:], start=True, stop=True)
        fbo1 = singles.tile([F, 1], F32, tag="fbo1")
        nc.vector.tensor_scalar(
            out=fbo1[:], in0=pbo1[:], scalar1=sbo1[:], scalar2=None, op0=ALU.add
        )

        # LN bn-stats into a shared mv view (cols of a [128, 8, 2] tile)
        def ln_bn(psum4, nch, mv_view, tag):
            for c in range(nch):
                st6 = small.tile([128, 6], F32, tag=f"st{tag}", name="st6")
                nc.vector.bn_stats(out=st6[:], in_=psum4[:, c, :])
                nc.vector.bn_aggr(out=mv_view[:, c, :], in_=st6[:])

        # combined fast-inverse-sqrt over a [128, n, 2] mv tile -> rstd/negms
        def fisr_n(mvc, n, tag, iters=1):
            a = small.tile([128, n], F32, tag=f"a{tag}", name="a")
            nc.vector.tensor_scalar(
                out=a[:], in0=mvc[:, :, 1], scalar1=EPS, scalar2=None, op0=ALU.add
            )
            bi = small.tile([128, n], I32, tag=f"bi{tag}", name="bi")
            nc.vector.tensor_scalar(
                out=bi[:], in0=a[:].bitcast(I32), scalar1=1, scalar2=None,
                op0=ALU.arith_shift_right,
            )
            y0 = small.tile([128, n], F32, tag=f"y0{tag}", name="y0")
            nc.vector.tensor_scalar(
                out=y0[:].bitcast(I32), in0=bi[:], scalar1=-1, scalar2=MAGIC,
                op0=ALU.mult, op1=ALU.add,
            )
            ha = small.tile([128, n], F32, tag=f"ha{tag}", name="ha")
            nc.vector.tensor_scalar(
                out=ha[:], in0=a[:], scalar1=-0.5, scalar2=None, op0=ALU.mult
            )
            y = y0
            for it in range(iters):
                yy = small.tile([128, n], F32, tag=f"yy{tag}{it}", name="yy")
                nc.vector.tensor_tensor(out=yy[:], in0=y[:], in1=y[:], op=ALU.mult)
                hyy = small.tile([128, n], F32, tag=f"hy{tag}{it}", name="hyy")
                nc.vector.tensor_tensor(out=hyy[:], in0=yy[:], in1=ha[:], op=ALU.mult)
                yn = small.tile([128, n], F32, tag=f"yn{tag}{it}", name="yn")
                nc.vector.scalar_tensor_tensor(
                    out=yn[:], in0=hyy[:], scalar=1.5, in1=y[:],
                    op0=ALU.add, op1=ALU.mult,
                )
                y = yn
            negms = small.tile([128, n], F32, tag=f"nm{tag}", name="negms")
            nc.vector.scalar_tensor_tensor(
                out=negms[:], in0=mvc[:, :, 0], scalar=-1.0, in1=y[:],
                op0=ALU.mult, op1=ALU.mult,
            )
            return y, negms

        # LayerNorm stats helper: psum4 [128, nch, 128] -> (mv, rstd, negms)
        #   mv[:, c, 0] = mean, rstd = 1/sqrt(var+eps), negms = -mean*rstd
        # bn runs on vector (PSUM reads); the rsqrt Newton chain runs on the
        # otherwise-idle gpsimd engine (SBUF-only tiles).
        def ln_stats(psum4, nch, tag, iters=1):
            mv = small.tile([128, nch, 2], F32, tag=f"mv{tag}")
            for c in range(nch):
                st6 = small.tile([128, 6], F32, tag=f"st{tag}")
                nc.vector.bn_stats(out=st6[:], in_=psum4[:, c, :])
                nc.vector.bn_aggr(out=mv[:, c, :], in_=st6[:])
            a = small.tile([128, nch], F32, tag=f"a{tag}")
            nc.vector.tensor_scalar(
                out=a[:], in0=mv[:, :, 1], scalar1=EPS, scalar2=None, op0=ALU.add
            )
            bi = small.tile([128, nch], I32, tag=f"bi{tag}")
            nc.vector.tensor_scalar(
                out=bi[:], in0=a[:].bitcast(I32), scalar1=1, scalar2=None,
                op0=ALU.arith_shift_right,
            )
            y0 = small.tile([128, nch], F32, tag=f"y0{tag}")
            nc.vector.tensor_scalar(
                out=y0[:].bitcast(I32), in0=bi[:], scalar1=-1, scalar2=MAGIC,
                op0=ALU.mult, op1=ALU.add,
            )
            ha = small.tile([128, nch], F32, tag=f"ha{tag}")
            nc.vector.tensor_scalar(
                out=ha[:], in0=a[:], scalar1=-0.5, scalar2=None, op0=ALU.mult
            )
            y = y0
            for it in range(iters):
                yy = small.tile([128, nch], F32, tag=f"yy{tag}{it}")
                nc.vector.tensor_tensor(out=yy[:], in0=y[:], in1=y[:], op=ALU.mult)
                hyy = small.tile([128, nch], F32, tag=f"hy{tag}{it}")
                nc.vector.tensor_tensor(out=hyy[:], in0=yy[:], in1=ha[:], op=ALU.mult)
                yn = small.tile([128, nch], F32, tag=f"yn{tag}{it}")
                nc.vector.scalar_tensor_tensor(
                    out=yn[:], in0=hyy[:], scalar=1.5, in1=y[:],
                    op0=ALU.add, op1=ALU.mult,
                )
                y = yn
            negms = small.tile([128, nch], F32, tag=f"nm{tag}")
            nc.vector.scalar_tensor_tensor(
                out=negms[:], in0=mv[:, :, 0], scalar=-1.0, in1=y[:],
                op0=ALU.mult, op1=ALU.mult,
            )
            return mv, y, negms

        # ---------- edge phase (4-stage software pipeline over groups) ------
        # S0: DMA loads   S1: embed MLP + LN-e stats   S2: apply-e, transpose,
        # update MLP + LN-u stats   S3: apply-u, one-hot aggregation.
        # Stages of group g are issued in different loop iterations so every
        # engine's program interleaves adjacent groups (no head-of-line
        # blocking on cross-engine dependencies).

        def edge_s0(g):
            # loads groups g and g+1 in one DMA per stream (g is even)
            hi = min(g + 2, NG)
            w = hi - g
            esl = slice(g * EPG, hi * EPG)
            t = {}
            t["efg"] = ld.tile([EIN, 2 * EPG], BF16, tag="efg", name="efg")
            nc.sync.dma_start(out=t["efg"][:, : w * EPG], in_=efT[:, esl])
            t["sfg"] = ld.tile([128, 2 * EPG], BF16, tag="sfg", name="sfg")
            nc.sync.dma_start(out=t["sfg"][:, : w * EPG], in_=sfT_d[:, esl])
            t["rfg"] = ld.tile([128, 2 * EPG], BF16, tag="rfg", name="rfg")
            nc.sync.dma_start(out=t["rfg"][:, : w * EPG], in_=rfT_d[:, esl])
            t["ohg"] = ld.tile([128, 2 * NC, G], BF16, tag="ohg", name="ohg")
            nc.sync.dma_start(
                out=t["ohg"][:, : w * NC, :],
                in_=oh_d[:, g * NC * G : hi * NC * G],
            )
            return t

        def edge_s1(g, t):
            h = g % 2
            pz1 = psB.tile([128, EPG], F32, tag="pB")
            nc.tensor.matmul(
                pz1[:], lhsT=sWe1[:],
                rhs=t["efg"][:, h * EPG : (h + 1) * EPG], start=True, stop=True,
            )
            y1 = work.tile([128, EPG], BF16, tag="y1")
            nc.scalar.activation(y1[:], pz1[:], AF.Silu, bias=sbe1[:], scale=1.0)

            pz2 = psA.tile([128, NC, 128], F32, tag="pA")
            if not skip_bias:
                nc.tensor.matmul(
                    pz2[:, :, :], lhsT=ones_r[:, :128], rhs=sbe2[:],
                    start=True, stop=False, skip_group_check=True,
                )
            for c in range(NC):
                nc.tensor.matmul(
                    pz2[:, c, :],
                    lhsT=y1[:, c * 128 : (c + 1) * 128],
                    rhs=sWe2[:],
                    start=skip_bias,
                    stop=skip_bias or (c == NC - 1),
                    skip_group_check=True,
                )
            t["pz2"] = pz2
            mve = small.tile([128, 4, 2], F32, tag="mve", name="mve")
            ln_bn(pz2, NC, mve[:, :, :], "e")
            t["mve"] = mve
            t["fe"] = fisr_n(mve, 4, "e")

        def edge_s2(g, t):
            pz2 = t["pz2"]
            rs8, nm8 = t["fe"]
            ln1 = work.tile([128, NC, 128], BF16, tag="ln1")
            for c in range(NC):
                nc.scalar.activation(
                    ln1[:, c, :], pz2[:, c, :], AF.Identity,
                    bias=nm8[:, c : c + 1], scale=rs8[:, c : c + 1],
                )
            t["ln1"] = ln1

            ptr = psS.tile([128, NC, 128], BF16, tag="pS")
            for c in range(NC):
                nc.tensor.transpose(ptr[:, c, :], ln1[:, c, :], ident16[:])
            ln1T = work.tile([128, NC // 2, 2, 128], BF16, tag="ln1T")
            nc.vector.tensor_copy(out=ln1T[:, :, 0, :], in_=ptr[:, 0::2, :])
            nc.scalar.activation(ln1T[:, :, 1, :], ptr[:, 1::2, :], AF.Copy)

            pu1 = psB.tile([128, EPG], F32, tag="pB")
            nc.tensor.matmul(
                pu1[:], lhsT=fWu1a[:], rhs=ln1T[:],
                start=True, stop=False,
            )
            h = g % 2
            nc.tensor.matmul(
                pu1[:], lhsT=sWu1b[:],
                rhs=t["sfg"][:, h * EPG : (h + 1) * EPG],
                start=False, stop=False,
            )
            nc.tensor.matmul(
                pu1[:], lhsT=sWu1c[:],
                rhs=t["rfg"][:, h * EPG : (h + 1) * EPG],
                start=False, stop=True,
            )
            yu = work.tile([128, EPG], BF16, tag="yu")
            nc.scalar.activation(yu[:], pu1[:], AF.Silu, bias=fbu1[:], scale=1.0)

            pu2 = psA.tile([128, NC, 128], F32, tag="pA")
            if not skip_bias:
                nc.tensor.matmul(
                    pu2[:, :, :], lhsT=ones_r[:, :128], rhs=sbu2[:],
                    start=True, stop=False, skip_group_check=True,
                )
            for c in range(NC):
                nc.tensor.matmul(
                    pu2[:, c, :],
                    lhsT=yu[:, c * 128 : (c + 1) * 128],
                    rhs=sWu2[:],
                    start=skip_bias,
                    stop=skip_bias or (c == NC - 1),
                    skip_group_check=True,
                )
            t["pu2"] = pu2
            mvu = small.tile([128, 4, 2], F32, tag="mvu", name="mvu")
            ln_bn(pu2, NC, mvu[:, :, :], "u")
            t["mvu"] = mvu
            t["fu"] = fisr_n(mvu, 4, "u")

        def edge_s3(g, t):
            pu2 = t["pu2"]
            rs8, nm8 = t["fu"]
            mvu = t["mvu"]
            ln1 = t["ln1"]
            ln2 = work.tile([128, NC, 128], BF16, tag="ln2")
            for c in range(NC):
                if c % 2 == 0:
                    nc.scalar.activation(
                        ln2[:, c, :], pu2[:, c, :], AF.Identity,
                        bias=nm8[:, c : c + 1], scale=rs8[:, c : c + 1],
                    )
                else:
                    nc.vector.tensor_scalar(
                        out=ln2[:, c, :], in0=pu2[:, c, :],
                        scalar1=mvu[:, c, 0:1], scalar2=rs8[:, c : c + 1],
                        op0=ALU.subtract, op1=ALU.mult,
                    )

            h = g % 2
            Sps = psS.tile([128, 2 * G], F32, tag="pS")
            for c in range(NC):
                nc.tensor.matmul(
                    Sps[:, 0:G], lhsT=ln1[:, c, :],
                    rhs=t["ohg"][:, h * NC + c, :],
                    start=(c == 0), stop=(c == NC - 1),
                )
            for c in range(NC):
                nc.tensor.matmul(
                    Sps[:, G : 2 * G], lhsT=ln2[:, c, :],
                    rhs=t["ohg"][:, h * NC + c, :],
                    start=(c == 0), stop=(c == NC - 1),
                )
            nc.vector.tensor_copy(
                out=aggSB1[:, g * G : (g + 1) * G], in_=Sps[:, 0:G]
            )
            nc.vector.tensor_copy(
                out=aggSB2[:, g * G : (g + 1) * G], in_=Sps[:, G : 2 * G]
            )

        # ---------- pnode phase (2-stage software pipeline over blocks) -----
        def pn_s1(j):
            sl = slice(j * PB, (j + 1) * PB)
            t = {"sl": sl}
            pn16b = ld.tile([128, PB], BF16, tag="pn16b")
            nc.sync.dma_start(out=pn16b[:], in_=pnT_d[:, sl])
            t["pn16b"] = pn16b

            pzp = psB.tile([128, PB], F32, tag="pB")
            nc.tensor.matmul(pzp[:], lhsT=sWp1n[:], rhs=pn16b[:], start=True, stop=False)
            nc.tensor.matmul(
                pzp[:], lhsT=fWp1ge[:], rhs=aggSB1[:, sl], start=False, stop=False
            )
            nc.tensor.matmul(
                pzp[:], lhsT=fWp1gu[:], rhs=aggSB2[:, sl], start=False, stop=False
            )
            nc.tensor.matmul(
                pzp[:], lhsT=bpe_row[:], rhs=sm01[:, sl], start=False, stop=True
            )
            yp = work.tile([128, PB], BF16, tag="yu")
            nc.scalar.activation(yp[:], pzp[:], AF.Silu, bias=sbp1[:], scale=1.0)

            pp2 = psA.tile([128, NC, 128], F32, tag="pA")
            if not skip_bias:
                nc.tensor.matmul(
                    pp2[:, :, :], lhsT=ones_r[:, :128], rhs=sbp2[:],
                    start=True, stop=False, skip_group_check=True,
                )
            for c in range(NC):
                nc.tensor.matmul(
                    pp2[:, c, :],
                    lhsT=yp[:, c * 128 : (c + 1) * 128],
                    rhs=sWp2[:],
                    start=skip_bias,
                    stop=skip_bias or (c == NC - 1),
                    skip_group_check=True,
                )
            t["pp2"] = pp2
            mvp = small.tile([128, 4, 2], F32, tag="mvp", name="mvp")
            ln_bn(pp2, NC, mvp[:, :, :], "p")
            t["mvp"] = mvp
            t["fp"] = fisr_n(mvp, 4, "p")
            return t

        def pn_s2(j, t):
            sl = t["sl"]
            pp2 = t["pp2"]
            rsp, nmp = t["fp"]
            lnp = work.tile([128, NC, 128], BF16, tag="ln1")
            for c in range(NC):
                nc.scalar.activation(
                    lnp[:, c, :], pp2[:, c, :], AF.Identity,
                    bias=nmp[:, c : c + 1], scale=rsp[:, c : c + 1],
                )

            ptr2 = psS.tile([128, NC, 128], BF16, tag="pS")
            for c in range(NC):
                nc.tensor.transpose(ptr2[:, c, :], lnp[:, c, :], ident16[:])
            lnpT = work.tile([128, NC // 2, 2, 128], BF16, tag="ln1T")
            nc.vector.tensor_copy(out=lnpT[:, :, 0, :], in_=ptr2[:, 0::2, :])
            nc.scalar.activation(lnpT[:, :, 1, :], ptr2[:, 1::2, :], AF.Copy)

            pzo = psB.tile([128, PB], F32, tag="pB")
            nc.tensor.matmul(
                pzo[:], lhsT=fWo1[:], rhs=lnpT[:],
                start=True, stop=False,
            )
            nc.tensor.matmul(
                pzo[:], lhsT=sWo116[:], rhs=t["pn16b"][:], start=False, stop=True
            )
            yo = work.tile([128, PB], BF16, tag="ln2")
            nc.scalar.activation(yo[:], pzo[:], AF.Silu, bias=fbo1[:], scale=1.0)

            po = psS.tile([OUT, PB], F32, tag="pS")
            nc.tensor.matmul(po[:], lhsT=sWo2[:], rhs=yo[:], start=True, stop=False)
            nc.tensor.matmul(po[:], lhsT=sbo2[:], rhs=ones_r[:], start=False, stop=True)
            oc = work.tile([OUT, PB], F32, tag="oc")
            nc.vector.tensor_copy(out=oc[:], in_=po[:])
            nc.sync.dma_start(out=outT[:, sl], in_=oc[:])

        state = {}
        shared = {}
        pn_ready = []
        pn_issued = []
        pn_last = {}
        next_pn = 0
        for i in range(NG + 3):
            if i < NG and i % 2 == 0:
                shared[i] = edge_s0(i)
                state[i] = dict(shared[i])
                if i + 1 < NG:
                    state[i + 1] = dict(shared[i])
            if 0 <= i - 1 < NG:
                edge_s1(i - 1, state[i - 1])
            if 0 <= i - 2 < NG:
                edge_s2(i - 2, state[i - 2])
            if 0 <= i - 3 < NG:
                edge_s3(i - 3, state[i - 3])
                del state[i - 3]
                g3 = i - 3
                while (
                    next_pn < NPB
                    and ((next_pn + 1) * PB - 1) // G <= g3
                ):
                    pn_ready.append(next_pn)
                    next_pn += 1
                if pn_issued and pn_issued[0][0] <= g3 - 2:
                    j0, t0 = pn_issued.pop(0)
                    pn_s2(j0, t0)
                if pn_ready and g3 % 2 == 0:
                    j0 = pn_ready.pop(0)
                    pn_issued.append((g3, pn_s1(j0), ))
                    pn_issued[-1] = (g3, pn_issued[-1][1])
                    pn_last[j0] = pn_issued[-1][1]


        # flush: finish any pnode blocks not yet issued/completed
        for j0, t0 in pn_issued:
            pn_s2(j0, t0)
        pstate = {}
        rest = list(pn_ready) + list(range(next_pn, NPB))
        for k, j in enumerate(rest):
            pstate[j] = pn_s1(j)
            if k >= 1:
                jp = rest[k - 1]
                pn_s2(jp, pstate.pop(jp))
        if rest:
            pn_s2(rest[-1], pstate.pop(rest[-1]))

    nc.compile()
    return nc


def _prep_core(ef_b, snd_b, rcv_b, rn_b, pn_b, tau_b, q):
    import ml_dtypes

    lo = q * QP
    mask = (rcv_b >= lo) & (rcv_b < lo + QP)
    ed = np.nonzero(mask)[0]
    loc = (rcv_b[ed] - lo).astype(np.int64)
    order = np.argsort(loc, kind="stable")
    ed, loc = ed[order], loc[order]
    grp = loc // G
    cnts = np.bincount(grp, minlength=NG)
    assert cnts.max() <= EPG, f"group overflow: {cnts.max()} > {EPG}"
    gstart = np.concatenate([[0], np.cumsum(cnts)[:-1]])
    slot = grp * EPG + (np.arange(len(ed)) - gstart[grp])

    efp = np.zeros((NEP, EIN), np.float32)
    efp[slot] = ef_b[ed]
    sf = np.zeros((NEP, F), np.float32)
    sf[slot] = rn_b[snd_b[ed]]
    rf = np.zeros((NEP, F), np.float32)
    rf[slot] = pn_b[lo + rcv_b[ed] - lo]
    cnt_all = np.bincount(loc, minlength=QP).astype(np.float32)
    ohf = np.zeros((NEP, G), np.float32)
    ohf[slot, loc - grp * G] = 1.0 / cnt_all[loc]
    oh_dev = np.ascontiguousarray(
        ohf.reshape(NG, NC, 128, G).transpose(2, 0, 1, 3).reshape(128, NG * NC * G)
    )

    m01_seg = np.minimum(cnt_all, 1.0)

    pn_q = pn_b[lo : lo + QP]
    bf = ml_dtypes.bfloat16
    return {
        "m01": m01_seg.reshape(1, QP).astype(bf),
        "efT": np.ascontiguousarray(efp.T.astype(bf)),
        "sfT": np.ascontiguousarray(sf.T.astype(bf)),
        "rfT": np.ascontiguousarray(rf.T.astype(bf)),
        "oh": oh_dev.astype(bf),
        "pnT": np.ascontiguousarray(pn_q.T.astype(bf)),
        "tau": tau_b.reshape(1, 1).astype(np.float32),
    }


def _prep_weights(i):
    w = {
        "We1": i["We1"], "be1": i["be1"].reshape(F, 1), "We2": i["We2"],
        "be2": np.tile(i["be2"].reshape(1, F), (1, NC)),
        "Wu1a": i["Wu1"][0:F], "Wu1b": i["Wu1"][F : 2 * F],
        "Wu1c": i["Wu1"][2 * F : 3 * F],
        "bu1": i["bu1"].reshape(F, 1), "Wu2": i["Wu2"],
        "bu2": np.tile(i["bu2"].reshape(1, F), (1, NC)),
        "Wp1n": i["Wp1"][0:F], "Wp1g": i["Wp1"][F : 2 * F],
        "bp1": i["bp1"].reshape(F, 1), "Wp2": i["Wp2"],
        "bp2": np.tile(i["bp2"].reshape(1, F), (1, NC)),
        "Wo1": i["Wo1"], "bo1": i["bo1"].reshape(F, 1), "Wo2": i["Wo2"],
        "bo2": i["bo2"].reshape(1, OUT),
    }
    for k in ("e", "u", "p"):
        C1, c1 = i[f"C{k}1"], i[f"c{k}1"]
        C2, c2 = i[f"C{k}2"], i[f"c{k}2"]
        w[f"C{k}1"] = C1.reshape(1, H)
        w[f"c{k}1"] = c1.reshape(H, 1)
        w[f"C{k}2a"] = np.ascontiguousarray(C2[:, 0:F])
        w[f"C{k}2b"] = np.ascontiguousarray(C2[:, F : 2 * F])
        w[f"c{k}2a1"] = (c2[0:F] + 1.0).reshape(F, 1)
        w[f"c{k}2b"] = c2[F : 2 * F].reshape(F, 1)
    import ml_dtypes

    bf16_keys = {"We1", "We2", "Wu1b", "Wu1c", "Wu2", "Wp1n", "Wp2", "Wo2",
                 "be2", "bu2", "bp2", "bo2"}
    return {
        k: np.ascontiguousarray(
            v, dtype=ml_dtypes.bfloat16 if k in bf16_keys else np.float32
        )
        for k, v in w.items()
    }


_NC_CACHE = {}


def _all_bias_zero(i):
    return all(
        not np.any(np.asarray(i[k]))
        for k in ("be2", "bu2", "bp2")
    )


def build_in_maps(inputs):
    i = {k: np.asarray(v) for k, v in inputs.items()}
    w = _prep_weights(i)
    in_maps = []
    for core in range(8):
        b, q = core // NQ, core % NQ
        m = dict(w)
        m.update(
            _prep_core(
                i["edge_features"][b], i["senders"][b], i["receivers"][b],
                i["rnode_features"][b], i["pnode_features"][b], i["tau"][b], q
            )
        )
        in_maps.append(m)
    return in_maps


def get_nc(skip_bias=False):
    key = ("nc", bool(skip_bias))
    if key not in _NC_CACHE:
        _NC_CACHE[key] = _build_nc(skip_bias=skip_bias)
    return _NC_CACHE[key]


def assemble(results):
    out = np.zeros((B, NPTOT, OUT), np.float32)
    for core in range(8):
        b, q = core // NQ, core % NQ
        out[b, q * QP : (q + 1) * QP, :] = results[core]["outT"].T
    return out


def kernel(**inputs):
    from concourse.bass_utils import run_bass_kernel_spmd

    nc = get_nc(skip_bias=_all_bias_zero(inputs))
    in_maps = build_in_maps(inputs)
    res = run_bass_kernel_spmd(nc, in_maps, list(range(8)))
    return assemble(res.results)


if __name__ == "__main__":
    import reference

    inputs = reference.setup_inputs()
    out = kernel(**{k: np.asarray(v) for k, v in inputs.items()})
    print("out", out.shape, out.dtype)
